# revision 21
# baseline (speedup 1.0000x reference)
"""Multi-head GAT Bass kernel for 8 Trainium2 NeuronCores (fp8 edge pipeline).

Sharding: destination-node row-parallel (24 global blocks of 128 rows; core c
owns blocks 3c..3c+2 = 384 output rows). Edges bucketed by destination block
on the host, padded to a uniform per-block count (256-multiple). No
collectives; the host concatenates per-core outputs.

Per-core pipeline (all per-edge tensors fp8):
  stage0: b/a/v projection tables via fp8 DoubleRow matmuls (x, W in fp8,
          weights pre-scaled by 16 for fp8 range; psum f32, cast to fp8).
  edges:  transposed fp8 gathers of a[e0-local] and b[e1] into one tile;
          z = a+b via ONE DoubleRow matmul per slot (identity-pair k-tiles);
          leaky-relu on Act engine (PSUM -> fp8 SBUF) with a parity-
          deinterleaving output AP (the 16-bit-granularity transpose of the
          fp8 gather interleaves features at byte level);
          per-head dot with Wa via J-accumulated DoubleRow matmuls;
          exp on Act; PE transpose; payload p*v on DVE; onehot accumulation
          into psum via group-paired DoubleRow matmuls.
  post:   divide, ELU, residual, LayerNorm (as v1).
"""
import sys
sys.path.insert(0, '/opt/trn_rl_repo')

from contextlib import ExitStack

import numpy as np
import ml_dtypes

import concourse.bass as bass
import concourse.bacc as bacc
import concourse.tile as tile
from concourse import mybir
from concourse.bass_utils import run_bass_kernel_spmd

N = 3072
HID = 512
H = 8
HD = 64
E = 98304
LN_EPS = 1e-5
NCORES = 8
NBLK = 24
BPC = 3
R = 128 * BPC
WS = 16.0     # weight pre-scale for a/b tables (z stored x16)
WAS = 32.0    # wa pre-scale

f32 = mybir.dt.float32
bf16 = mybir.dt.bfloat16
fp8 = mybir.dt.float8e4
f8np = ml_dtypes.float8_e4m3fn
Alu = mybir.AluOpType
Act = mybir.ActivationFunctionType
DR = mybir.MatmulPerfMode.DoubleRow

# fraction of leaky-relu quarters on the DVE engine (rest on Act): k of 8
LEAKY_DVE_K = 0
# b-table evacuation: k of 4 halves on DVE (rest on Act)
BEVAC_DVE_K = 2


def _wrap_idx(idx):
    n = idx.shape[0]
    assert n % 16 == 0
    w16 = idx.reshape(n // 16, 16).T.astype(np.int16)
    return np.ascontiguousarray(np.tile(w16, (8, 1)))


def _ktile(a, k=4):
    """[K, M] -> [128, k, M] with row f = kk*128 + p."""
    K, M = a.shape
    assert K == 128 * k
    return np.ascontiguousarray(a.reshape(k, 128, M).transpose(1, 0, 2))


def prepare(x, edges, Wv, bv, Ww, bw, Wa, ba, gamma, beta):
    e0 = np.asarray(edges[0], np.int64) % N
    e1 = np.asarray(edges[1], np.int64) % N
    blk = e0 >> 7
    order = np.argsort(blk, kind="stable")
    counts = np.bincount(blk, minlength=NBLK)
    B_pad = max(256, int(-(-counts.max() // 256) * 256))
    P = BPC * B_pad

    ga_idx = np.zeros((NBLK, B_pad), np.int16)   # local (within-core) a rows
    gb_idx = np.zeros((NBLK, B_pad), np.int16)
    oh = np.zeros((NBLK, B_pad, 128), np.float32)
    starts = np.zeros(NBLK + 1, np.int64)
    starts[1:] = np.cumsum(counts)
    for b in range(NBLK):
        ids = order[starts[b]:starts[b + 1]]
        c = len(ids)
        core = b // BPC
        ga_idx[b, :c] = e0[ids] - 384 * core
        ga_idx[b, c:] = (b % BPC) * 128
        gb_idx[b, :c] = e1[ids]
        oh[b, np.arange(c), e0[ids] - b * 128] = 1.0

    x = np.asarray(x, np.float32)
    xT = np.ascontiguousarray(x.T)
    xt8 = _ktile(xT).astype(f8np)
    Ww = np.asarray(Ww, np.float32)
    wt8 = _ktile(Ww[:HID] * WS).astype(f8np)
    wb8 = _ktile(Ww[HID:] * WS).astype(f8np)
    wv8 = _ktile(np.asarray(Wv, np.float32)).astype(f8np)
    wa = np.asarray(Wa, np.float32).reshape(2 * HD)

    # wzf [128, 2, 32]: per J-slice [128, 2, 8]: col (2J+u) = wa[2*(p%64)+par]
    # for u == (p >= 64), else 0
    wzf = np.zeros((128, 2, 32), np.float32)
    pr = np.arange(128)
    for par in range(2):
        val = wa[2 * (pr % 64) + par] * WAS
        for J in range(4):
            u = (pr >= 64).astype(np.int64)
            wzf[pr, par, 8 * J + 2 * J + u] = val
    wzf8 = np.ascontiguousarray(wzf.astype(f8np).reshape(128, 64))

    id2 = np.zeros((128, 2, 128), np.float32)
    id2[pr, 0, pr] = 1.0
    id2[pr, 1, pr] = 1.0
    id2_8 = np.ascontiguousarray(id2.astype(f8np).reshape(128, 256))

    bwr = np.ascontiguousarray(np.broadcast_to(bw * WS / 128.0, (128, 2 * HID)).astype(ml_dtypes.bfloat16))
    bvr = np.ascontiguousarray(np.broadcast_to(np.asarray(bv, np.float32) / 128.0, (128, HID)).astype(ml_dtypes.bfloat16))
    gamma_b = np.ascontiguousarray(np.broadcast_to(gamma, (128, HID)).astype(np.float32))
    beta_b = np.ascontiguousarray(np.broadcast_to(beta, (128, HID)).astype(np.float32))
    ba_b = np.full((128, 1), float(np.asarray(ba).reshape(-1)[0]), np.float32)

    in_maps = []
    for c in range(NCORES):
        bs = slice(BPC * c, BPC * (c + 1))
        in_maps.append(dict(
            xt8=xt8,
            xt8o=np.ascontiguousarray(xt8[:, :, R * c:R * (c + 1)]),
            wb8=wb8, wt8=wt8, wv8=wv8,
            bwr=bwr, bvr=bvr,
            gamma_b=gamma_b, beta_b=beta_b, ba_b=ba_b,
            wzf=wzf8, id2=id2_8,
            xs=np.ascontiguousarray(x[R * c:R * (c + 1)]),
            gai=_wrap_idx(ga_idx[bs].reshape(-1)),
            gbi=_wrap_idx(gb_idx[bs].reshape(-1)),
            oh8=np.ascontiguousarray(
                oh[bs].reshape(BPC, B_pad // 128, 128, 128).transpose(2, 0, 1, 3)
                .reshape(128, BPC * (B_pad // 128) * 128).astype(f8np)),
        ))
    return in_maps, B_pad, P


def build(B_pad, P):
    nc = bacc.Bacc("TRN2", target_bir_lowering=False, num_devices=NCORES)

    xt8_in = nc.dram_tensor("xt8", [128, 4 * N], fp8, kind="ExternalInput").ap()
    xt8o_in = nc.dram_tensor("xt8o", [128, 4 * R], fp8, kind="ExternalInput").ap()
    wb8_in = nc.dram_tensor("wb8", [128, 4 * 2 * HID], fp8, kind="ExternalInput").ap()
    wt8_in = nc.dram_tensor("wt8", [128, 4 * 2 * HID], fp8, kind="ExternalInput").ap()
    wv8_in = nc.dram_tensor("wv8", [128, 4 * HID], fp8, kind="ExternalInput").ap()
    bwr_in = nc.dram_tensor("bwr", [128, 2 * HID], bf16, kind="ExternalInput").ap()
    bvr_in = nc.dram_tensor("bvr", [128, HID], bf16, kind="ExternalInput").ap()
    gam_in = nc.dram_tensor("gamma_b", [128, HID], f32, kind="ExternalInput").ap()
    bet_in = nc.dram_tensor("beta_b", [128, HID], f32, kind="ExternalInput").ap()
    ba_in = nc.dram_tensor("ba_b", [128, 1], f32, kind="ExternalInput").ap()
    wzf_in = nc.dram_tensor("wzf", [128, 64], fp8, kind="ExternalInput").ap()
    id2_in = nc.dram_tensor("id2", [128, 256], fp8, kind="ExternalInput").ap()
    xs_in = nc.dram_tensor("xs", [R, HID], f32, kind="ExternalInput").ap()
    gai_in = nc.dram_tensor("gai", [128, P // 16], mybir.dt.int16, kind="ExternalInput").ap()
    gbi_in = nc.dram_tensor("gbi", [128, P // 16], mybir.dt.int16, kind="ExternalInput").ap()
    oh8_in = nc.dram_tensor("oh8", [128, P], fp8, kind="ExternalInput").ap()
    y_out = nc.dram_tensor("y", [R, HID], f32, kind="ExternalOutput").ap()

    a_tbl = nc.dram_tensor("a_tbl", [R, 2 * HID], fp8, kind="Internal").ap()
    b_tbl = nc.dram_tensor("b_tbl", [N, 2 * HID], fp8, kind="Internal").ap()
    v_tbl = nc.dram_tensor("v_tbl", [N, HID], fp8, kind="Internal").ap()

    NT = N // 128
    G = B_pad // 128          # groups per block (even)

    with tile.TileContext(nc) as tc, ExitStack() as ctx:
        const = ctx.enter_context(tc.tile_pool(name="const", bufs=1))

        # f32 identity for PE transpose of p8
        iota_row = const.tile([128, 128], mybir.dt.int32)
        nc.gpsimd.iota(iota_row[:], pattern=[[1, 128]], base=0, channel_multiplier=0)
        pid = const.tile([128, 1], mybir.dt.int32)
        nc.gpsimd.iota(pid[:], pattern=[[0, 1]], base=0, channel_multiplier=1)
        iota_f = const.tile([128, 128], f32)
        nc.vector.tensor_copy(iota_f[:], iota_row[:])
        pid_f = const.tile([128, 1], f32)
        nc.vector.tensor_copy(pid_f[:], pid[:])
        ident = const.tile([128, 128], f32)
        nc.vector.tensor_scalar(ident[:], iota_f[:], pid_f[:], None, op0=Alu.is_equal)

        gam_sb = const.tile([128, HID], f32)
        nc.sync.dma_start(gam_sb[:], gam_in)
        bet_sb = const.tile([128, HID], f32)
        nc.sync.dma_start(bet_sb[:], bet_in)
        ba_sb = const.tile([128, 1], f32)
        nc.sync.dma_start(ba_sb[:], ba_in)
        wzf_sb = const.tile([128, 2, 32], fp8)
        nc.sync.dma_start(wzf_sb[:], wzf_in.rearrange("p (a b) -> p a b", a=2))
        id2_sb = const.tile([128, 2, 128], fp8)
        nc.sync.dma_start(id2_sb[:], id2_in.rearrange("p (a b) -> p a b", a=2))
        xs_sb = const.tile([128, BPC, HID], f32)
        nc.sync.dma_start(xs_sb[:], xs_in.rearrange("(b p) d -> p b d", p=128))
        gai_sb = const.tile([128, P // 16], mybir.dt.int16)
        nc.sync.dma_start(gai_sb[:], gai_in)
        gbi_sb = const.tile([128, P // 16], mybir.dt.int16)
        nc.sync.dma_start(gbi_sb[:], gbi_in)

        # ---------------- Stage 0: fp8 projection tables ----------------
        with ExitStack() as s0:
            wpool = s0.enter_context(tc.tile_pool(name="wpool", bufs=1))
            s0p = s0.enter_context(tc.tile_pool(name="s0p", bufs=3))
            psum_b = s0.enter_context(tc.tile_pool(name="psum_b", bufs=2, space="PSUM"))
            psum_a = s0.enter_context(tc.tile_pool(name="psum_a", bufs=1, space="PSUM"))
            psum_v = s0.enter_context(tc.tile_pool(name="psum_v", bufs=2, space="PSUM"))

            xt8_sb = wpool.tile([128, 4, N], fp8)
            nc.sync.dma_start(xt8_sb[:], xt8_in.rearrange("p (a n) -> p a n", a=4))
            xt8o_sb = wpool.tile([128, 4, R], fp8)
            nc.sync.dma_start(xt8o_sb[:], xt8o_in.rearrange("p (a n) -> p a n", a=4))
            wb8_sb = wpool.tile([128, 4, 2 * HID], fp8)
            nc.sync.dma_start(wb8_sb[:], wb8_in.rearrange("p (a n) -> p a n", a=4))
            wt8_sb = wpool.tile([128, 4, 2 * HID], fp8)
            nc.sync.dma_start(wt8_sb[:], wt8_in.rearrange("p (a n) -> p a n", a=4))
            wv8_sb = wpool.tile([128, 4, HID], fp8)
            nc.sync.dma_start(wv8_sb[:], wv8_in.rearrange("p (a n) -> p a n", a=4))
            bwr_sb = wpool.tile([128, 2 * HID], bf16)
            nc.sync.dma_start(bwr_sb[:], bwr_in)
            bvr_sb = wpool.tile([128, HID], bf16)
            nc.sync.dma_start(bvr_sb[:], bvr_in)
            ones1 = wpool.tile([128, 128], bf16)
            nc.vector.memset(ones1[:], 1.0)

            # a table first (tiny; gates ga gathers)
            a8 = s0p.tile([128, BPC, 2 * HID], fp8, tag="a8")
            for t in range(BPC):
                psa = psum_a.tile([128, 2 * HID], f32, tag="ps_a")
                for half in range(2):
                    hs = slice(half * HID, (half + 1) * HID)
                    nc.tensor.matmul(psa[:, hs], xt8o_sb[:, 0:2, t * 128:(t + 1) * 128],
                                     wt8_sb[:, 0:2, hs], start=True, stop=False, perf_mode=DR,
                                     skip_group_check=True)
                    nc.tensor.matmul(psa[:, hs], xt8o_sb[:, 2:4, t * 128:(t + 1) * 128],
                                     wt8_sb[:, 2:4, hs], start=False, stop=False, perf_mode=DR,
                                     skip_group_check=True)
                    nc.tensor.matmul(psa[:, hs], ones1[:], bwr_sb[:, hs], start=False,
                                     stop=True, skip_group_check=True)
                    if (t + half) % 2 == 0:
                        nc.vector.tensor_copy(a8[:, t, hs], psa[:, hs])
                    else:
                        nc.scalar.copy(a8[:, t, hs], psa[:, hs])
            nc.sync.dma_start(a_tbl.rearrange("(t p) f -> p t f", p=128), a8[:])
            # b table (gates gb gathers); write in 2-tile batches
            evac = 0
            for nt2 in range(NT // 2):
                b8 = s0p.tile([128, 2, 2 * HID], fp8, tag="b8")
                for k in range(2):
                    nt = 2 * nt2 + k
                    for half in range(2):
                        hs = slice(half * HID, (half + 1) * HID)
                        psb = psum_b.tile([128, HID], f32, tag="ps_b")
                        nc.tensor.matmul(psb[:], xt8_sb[:, 0:2, nt * 128:(nt + 1) * 128],
                                         wb8_sb[:, 0:2, hs], start=True, stop=False, perf_mode=DR)
                        nc.tensor.matmul(psb[:], xt8_sb[:, 2:4, nt * 128:(nt + 1) * 128],
                                         wb8_sb[:, 2:4, hs], start=False, stop=True, perf_mode=DR)
                        if evac % 4 < BEVAC_DVE_K:
                            nc.vector.tensor_copy(b8[:, k, hs], psb[:])
                        else:
                            nc.scalar.copy(b8[:, k, hs], psb[:])
                        evac += 1
                nc.sync.dma_start(
                    b_tbl[nt2 * 256:(nt2 + 1) * 256, :].rearrange("(t p) f -> p t f", p=128),
                    b8[:])
            # v table, 2-tile write batches
            for nt2 in range(NT // 2):
                v8 = s0p.tile([128, 2, HID], fp8, tag="v8")
                for k in range(2):
                    nt = 2 * nt2 + k
                    psv = psum_v.tile([128, HID], f32, tag="ps_v")
                    nc.tensor.matmul(psv[:], xt8_sb[:, 0:2, nt * 128:(nt + 1) * 128],
                                     wv8_sb[:, 0:2, :], start=True, stop=False, perf_mode=DR,
                                     skip_group_check=True)
                    nc.tensor.matmul(psv[:], xt8_sb[:, 2:4, nt * 128:(nt + 1) * 128],
                                     wv8_sb[:, 2:4, :], start=False, stop=False, perf_mode=DR,
                                     skip_group_check=True)
                    nc.tensor.matmul(psv[:], ones1[:], bvr_sb[:], start=False, stop=True,
                                     skip_group_check=True)
                    if nt % 2 == 0:
                        nc.vector.tensor_copy(v8[:, k, :], psv[:])
                    else:
                        nc.scalar.copy(v8[:, k, :], psv[:])
                nc.sync.dma_start(
                    v_tbl[nt2 * 256:(nt2 + 1) * 256, :].rearrange("(t p) f -> p t f", p=128),
                    v8[:])

        # ---------------- Edge stage ----------------
        s12 = ctx.enter_context(tc.tile_pool(name="s12", bufs=3))
        gvp = ctx.enter_context(tc.tile_pool(name="gvp", bufs=2))
        acc = ctx.enter_context(tc.tile_pool(name="acc", bufs=1, space="PSUM"))
        zqp = ctx.enter_context(tc.tile_pool(name="zqp", bufs=2, space="PSUM"))
        pswp = ctx.enter_context(tc.tile_pool(name="pswp", bufs=1, space="PSUM"))
        pstp = ctx.enter_context(tc.tile_pool(name="pstp", bufs=1, space="PSUM"))
        post = ctx.enter_context(tc.tile_pool(name="post", bufs=1))

        chunks = []
        off = 0
        while off < B_pad:
            c = min(512, B_pad - off)
            chunks.append((off, c))
            off += c

        for blk in range(BPC):
            psum_y = acc.tile([128, HID], f32, tag="psum_y")
            psum_d = acc.tile([128, H], f32, tag="psum_d")
            base = blk * B_pad

            gv_blk = gvp.tile([128, G, HID], fp8, tag="gv")
            oh_blk = gvp.tile([128, G, 128], fp8, tag="ohb")
            nc.sync.dma_start(
                oh_blk[:], oh8_in[:, blk * G * 128:(blk + 1) * G * 128].rearrange(
                    "p (g r) -> p g r", g=G))

            def finish(st):
                p8, coff, C, NG, first, last = st
                pay = s12.tile([128, NG, HID + H], fp8, tag="pay")
                pst = pstp.tile([128, 4, H], f32, tag="pst")
                for g in range(NG):
                    nc.tensor.transpose(pst[:, g, :], p8[:, g * 128:(g + 1) * 128],
                                        ident[:H, :H])
                nc.vector.tensor_copy(pay[:, :, HID:], pst[:, :NG, :])
                nc.vector.tensor_mul(
                    pay[:, :, :HID].rearrange("p c (h d) -> p c h d", h=H),
                    gv_blk[:, coff // 128:coff // 128 + NG, :].rearrange(
                        "p c (h d) -> p c h d", h=H),
                    pay[:, :, HID:].unsqueeze(3).broadcast_to([128, NG, H, HD]))
                oh_c = oh_blk[:, coff // 128:coff // 128 + NG, :]
                for t in range(NG // 2):
                    st_ = first and t == 0
                    sp = last and t == NG // 2 - 1
                    nc.tensor.matmul(psum_y[:], oh_c[:, 2 * t:2 * t + 2, :],
                                     pay[:, 2 * t:2 * t + 2, :HID],
                                     start=st_, stop=sp, perf_mode=DR,
                                     skip_group_check=True)
                    nc.tensor.matmul(psum_d[:], oh_c[:, 2 * t:2 * t + 2, :],
                                     pay[:, 2 * t:2 * t + 2, HID:],
                                     start=st_, stop=sp, perf_mode=DR,
                                     skip_group_check=True)

            def dot_exp(st):
                zl2, coff, C, CH, NG, first, last = st
                psw = pswp.tile([8, 2, CH], f32, tag="psw")
                for eh in range(2):
                    for J in range(4):
                        nc.tensor.matmul(
                            psw[:, eh, :], wzf_sb[:, :, 8 * J:8 * J + 8],
                            zl2[:, 2 * J + eh, :, :],
                            start=(J == 0), stop=(J == 3), perf_mode=DR)
                p8 = s12.tile([8, C], f32, tag="p8")
                nc.scalar.activation(p8[:], psw[:].rearrange("p a c -> p (a c)"),
                                     Act.Exp, bias=ba_sb[:8, :], scale=1.0 / (WS * WAS))
                return (p8, coff, C, NG, first, last)

            prev_dot = None
            prev_fin = None
            for ci, (coff, C) in enumerate(chunks):
                off = base + coff
                i0, i1 = off // 16, (off + C) // 16
                CH = C // 2            # edges per eh-half / cc positions
                NG = C // 128          # groups in chunk

                gab = s12.tile([128, 2, 8, C], fp8, tag="gab")
                nc.gpsimd.dma_gather(
                    out_ap=gab[:, 0], in_ap=a_tbl, idxs_ap=gai_sb[:, i0:i1],
                    num_idxs=C, num_idxs_reg=C, elem_size=2 * HID, transpose=True)
                nc.gpsimd.dma_gather(
                    out_ap=gab[:, 1], in_ap=b_tbl, idxs_ap=gbi_sb[:, i0:i1],
                    num_idxs=C, num_idxs_reg=C, elem_size=2 * HID, transpose=True)
                nv = (B_pad + 1023) // 1024
                if ci < nv:
                    v0 = ci * 1024
                    vc = min(1024, B_pad - v0)
                    nc.gpsimd.dma_gather(
                        out_ap=gv_blk[:, v0 // 128:(v0 + vc) // 128, :], in_ap=v_tbl,
                        idxs_ap=gbi_sb[:, (base + v0) // 16:(base + v0 + vc) // 16],
                        num_idxs=vc, num_idxs_reg=vc, elem_size=HID)

                # z assembly + leaky -> zl2 [128, 8, 2, CH]
                zl2 = s12.tile([128, 8, 2, CH], fp8, tag="zl2")
                nhalf = C // 256
                for sg in range(2):
                    for ch in range(nhalf):
                        zq = zqp.tile([128, 4, 256], f32, tag="zq")
                        for jj in range(4):
                            j = 4 * sg + jj
                            nc.tensor.matmul(
                                zq[:, jj, :], id2_sb[:],
                                gab[:, :, j, ch * 256:(ch + 1) * 256],
                                start=True, stop=True, perf_mode=DR)
                        dst = zl2[:, 4 * sg:4 * sg + 4, :, ch * 128:(ch + 1) * 128]
                        dst = dst.rearrange("p s par cc -> p s cc par")
                        src = zq[:].rearrange("p s (cc par) -> p s cc par", par=2)
                        nc.scalar.activation(dst, src, Act.Prelu, alpha=0.01)

                if prev_dot is not None:
                    new_fin = dot_exp(prev_dot)
                    if prev_fin is not None:
                        finish(prev_fin)
                    prev_fin = new_fin
                prev_dot = (zl2, coff, C, CH, NG, ci == 0, ci == len(chunks) - 1)

            new_fin = dot_exp(prev_dot)
            if prev_fin is not None:
                finish(prev_fin)
            finish(new_fin)

            # ---------------- post: divide, ELU, residual, LayerNorm ----------------
            den = post.tile([128, H], f32, tag="den")
            nc.vector.tensor_scalar_add(den[:], psum_d[:], 1e-30)
            rden = post.tile([128, H], f32, tag="rden")
            nc.vector.reciprocal(rden[:], den[:])
            y1 = post.tile([128, HID], f32, tag="y1")
            nc.vector.tensor_mul(
                y1[:].rearrange("p (h d) -> p h d", h=H),
                psum_y[:].rearrange("p (h d) -> p h d", h=H),
                rden[:].unsqueeze(2).broadcast_to([128, H, HD]))
            m1 = post.tile([128, HID], f32, tag="m1")
            nc.vector.tensor_scalar_max(m1[:], y1[:], 0.0)
            t1 = post.tile([128, HID], f32, tag="t1")
            nc.vector.tensor_scalar_min(t1[:], y1[:], 0.0)
            t2 = post.tile([128, HID], f32, tag="t2")
            nc.scalar.activation(t2[:], t1[:], Act.Exp)
            y3 = post.tile([128, HID], f32, tag="y3")
            nc.vector.scalar_tensor_tensor(y3[:], t2[:], -1.0, m1[:],
                                           op0=Alu.add, op1=Alu.add)
            nc.vector.tensor_add(y3[:], y3[:], xs_sb[:, blk, :])
            mu = post.tile([128, 1], f32, tag="mu")
            nc.vector.reduce_sum(mu[:], y3[:], axis=mybir.AxisListType.X)
            nc.vector.tensor_scalar_mul(mu[:], mu[:], 1.0 / HID)
            yc = post.tile([128, HID], f32, tag="yc")
            nc.vector.tensor_scalar(yc[:], y3[:], mu[:], None, op0=Alu.subtract)
            sq = post.tile([128, HID], f32, tag="sq")
            nc.vector.tensor_mul(sq[:], yc[:], yc[:])
            s2 = post.tile([128, 1], f32, tag="s2")
            nc.vector.reduce_sum(s2[:], sq[:], axis=mybir.AxisListType.X)
            var = post.tile([128, 1], f32, tag="var")
            nc.vector.tensor_scalar(var[:], s2[:], 1.0 / HID, LN_EPS,
                                    op0=Alu.mult, op1=Alu.add)
            lnv = post.tile([128, 1], f32, tag="lnv")
            nc.scalar.activation(lnv[:], var[:], Act.Ln)
            rstd = post.tile([128, 1], f32, tag="rstd")
            nc.scalar.activation(rstd[:], lnv[:], Act.Exp, scale=-0.5)
            yn = post.tile([128, HID], f32, tag="yn")
            nc.vector.tensor_scalar(yn[:], yc[:], rstd[:], None, op0=Alu.mult)
            yf = post.tile([128, HID], f32, tag="yf")
            nc.vector.tensor_mul(yf[:], yn[:], gam_sb[:])
            nc.vector.tensor_add(yf[:], yf[:], bet_sb[:])
            nc.sync.dma_start(y_out[blk * 128:(blk + 1) * 128, :], yf[:])

    nc.compile()
    return nc


_CACHE = {}


def get_nc(B_pad, P):
    key = (B_pad, P)
    if key not in _CACHE:
        _CACHE[key] = build(B_pad, P)
    return _CACHE[key]


def kernel(**inputs) -> np.ndarray:
    in_maps, B_pad, P = prepare(**inputs)
    nc = get_nc(B_pad, P)
    res = run_bass_kernel_spmd(nc, in_maps, core_ids=list(range(NCORES)))
    out = np.concatenate([r["y"] for r in res.results], axis=0)
    return out.astype(np.float32)


if __name__ == "__main__":
    import reference
    inputs = {k: np.asarray(v) for k, v in reference.setup_inputs().items()}
    got = kernel(**inputs)
    want = np.asarray(reference.reference(**inputs))
    err = np.abs(got - want).max() / (np.abs(want).max() + 1e-12)
    print("abs-max relative error:", err)


# revision 25
# speedup vs baseline: 1.0017x; 1.0017x over previous
"""Multi-head GAT Bass kernel for 8 Trainium2 NeuronCores (fp8 edge pipeline).

Sharding: destination-node row-parallel (24 global blocks of 128 rows; core c
owns blocks 3c..3c+2 = 384 output rows). Edges bucketed by destination block
on the host, padded to a uniform per-block count (256-multiple). No
collectives; the host concatenates per-core outputs.

Per-core pipeline (all per-edge tensors fp8):
  stage0: b/a/v projection tables via fp8 DoubleRow matmuls (x, W in fp8,
          weights pre-scaled by 16 for fp8 range; psum f32, cast to fp8).
  edges:  transposed fp8 gathers of a[e0-local] and b[e1] into one tile;
          z = a+b via ONE DoubleRow matmul per slot (identity-pair k-tiles);
          leaky-relu on Act engine (PSUM -> fp8 SBUF) with a parity-
          deinterleaving output AP (the 16-bit-granularity transpose of the
          fp8 gather interleaves features at byte level);
          per-head dot with Wa via J-accumulated DoubleRow matmuls;
          exp on Act; PE transpose; payload p*v on DVE; onehot accumulation
          into psum via group-paired DoubleRow matmuls.
  post:   divide, ELU, residual, LayerNorm (as v1).
"""
import sys
sys.path.insert(0, '/opt/trn_rl_repo')

from contextlib import ExitStack

import numpy as np
import ml_dtypes

import concourse.bass as bass
import concourse.bacc as bacc
import concourse.tile as tile
from concourse import mybir
from concourse.bass_utils import run_bass_kernel_spmd

N = 3072
HID = 512
H = 8
HD = 64
E = 98304
LN_EPS = 1e-5
NCORES = 8
NBLK = 24
BPC = 3
R = 128 * BPC
WS = 16.0     # weight pre-scale for a/b tables (z stored x16)
WAS = 32.0    # wa pre-scale

f32 = mybir.dt.float32
bf16 = mybir.dt.bfloat16
fp8 = mybir.dt.float8e4
f8np = ml_dtypes.float8_e4m3fn
Alu = mybir.AluOpType
Act = mybir.ActivationFunctionType
DR = mybir.MatmulPerfMode.DoubleRow

# fraction of leaky-relu quarters on the DVE engine (rest on Act): k of 8
LEAKY_DVE_K = 0
# b-table evacuation: k of 4 halves on DVE (rest on Act)
BEVAC_DVE_K = 2


def _wrap_idx(idx):
    n = idx.shape[0]
    assert n % 16 == 0
    w16 = idx.reshape(n // 16, 16).T.astype(np.int16)
    return np.ascontiguousarray(np.tile(w16, (8, 1)))


def _ktile(a, k=4):
    """[K, M] -> [128, k, M] with row f = kk*128 + p."""
    K, M = a.shape
    assert K == 128 * k
    return np.ascontiguousarray(a.reshape(k, 128, M).transpose(1, 0, 2))


def prepare(x, edges, Wv, bv, Ww, bw, Wa, ba, gamma, beta):
    e0 = np.asarray(edges[0], np.int64) % N
    e1 = np.asarray(edges[1], np.int64) % N
    blk = e0 >> 7
    order = np.argsort(blk, kind="stable")
    counts = np.bincount(blk, minlength=NBLK)
    B_pad = max(256, int(-(-counts.max() // 256) * 256))
    P = BPC * B_pad

    ga_idx = np.zeros((NBLK, B_pad), np.int16)   # local (within-core) a rows
    gb_idx = np.zeros((NBLK, B_pad), np.int16)
    oh = np.zeros((NBLK, B_pad, 128), np.float32)
    starts = np.zeros(NBLK + 1, np.int64)
    starts[1:] = np.cumsum(counts)
    for b in range(NBLK):
        ids = order[starts[b]:starts[b + 1]]
        c = len(ids)
        core = b // BPC
        ga_idx[b, :c] = e0[ids] - 384 * core
        ga_idx[b, c:] = (b % BPC) * 128
        gb_idx[b, :c] = e1[ids]
        oh[b, np.arange(c), e0[ids] - b * 128] = 1.0

    x = np.asarray(x, np.float32)
    xT = np.ascontiguousarray(x.T)
    xt8 = _ktile(xT).astype(f8np)
    Ww = np.asarray(Ww, np.float32)
    wt8 = _ktile(Ww[:HID] * WS).astype(f8np)
    wb8 = _ktile(Ww[HID:] * WS).astype(f8np)
    wv8 = _ktile(np.asarray(Wv, np.float32)).astype(f8np)
    wa = np.asarray(Wa, np.float32).reshape(2 * HD)

    # wzf [128, 2, 32]: per J-slice [128, 2, 8]: col (2J+u) = wa[2*(p%64)+par]
    # for u == (p >= 64), else 0
    wzf = np.zeros((128, 2, 32), np.float32)
    pr = np.arange(128)
    for par in range(2):
        val = wa[2 * (pr % 64) + par] * WAS
        for J in range(4):
            u = (pr >= 64).astype(np.int64)
            wzf[pr, par, 8 * J + 2 * J + u] = val
    wzf8 = np.ascontiguousarray(wzf.astype(f8np).reshape(128, 64))

    id2 = np.zeros((128, 2, 128), np.float32)
    id2[pr, 0, pr] = 1.0
    id2[pr, 1, pr] = 1.0
    id2_8 = np.ascontiguousarray(id2.astype(f8np).reshape(128, 256))

    bwr = np.ascontiguousarray(np.broadcast_to(bw * WS / 128.0, (128, 2 * HID)).astype(ml_dtypes.bfloat16))
    bvr = np.ascontiguousarray(np.broadcast_to(np.asarray(bv, np.float32) / 128.0, (128, HID)).astype(ml_dtypes.bfloat16))
    gamma_b = np.ascontiguousarray(np.broadcast_to(gamma, (128, HID)).astype(np.float32))
    beta_b = np.ascontiguousarray(np.broadcast_to(beta, (128, HID)).astype(np.float32))
    ba_b = np.full((128, 1), float(np.asarray(ba).reshape(-1)[0]), np.float32)

    in_maps = []
    for c in range(NCORES):
        bs = slice(BPC * c, BPC * (c + 1))
        in_maps.append(dict(
            xt8=xt8,
            xt8o=np.ascontiguousarray(xt8[:, :, R * c:R * (c + 1)]),
            wb8=wb8, wt8=wt8, wv8=wv8,
            bwr=bwr, bvr=bvr,
            gamma_b=gamma_b, beta_b=beta_b, ba_b=ba_b,
            wzf=wzf8, id2=id2_8,
            xs=np.ascontiguousarray(x[R * c:R * (c + 1)]),
            gai=_wrap_idx(ga_idx[bs].reshape(-1)),
            gbi=_wrap_idx(gb_idx[bs].reshape(-1)),
            oh8=np.ascontiguousarray(
                oh[bs].reshape(BPC, B_pad // 128, 128, 128).transpose(2, 0, 1, 3)
                .reshape(128, BPC * (B_pad // 128) * 128).astype(f8np)),
        ))
    return in_maps, B_pad, P


def build(B_pad, P):
    nc = bacc.Bacc("TRN2", target_bir_lowering=False, num_devices=NCORES)

    xt8_in = nc.dram_tensor("xt8", [128, 4 * N], fp8, kind="ExternalInput").ap()
    xt8o_in = nc.dram_tensor("xt8o", [128, 4 * R], fp8, kind="ExternalInput").ap()
    wb8_in = nc.dram_tensor("wb8", [128, 4 * 2 * HID], fp8, kind="ExternalInput").ap()
    wt8_in = nc.dram_tensor("wt8", [128, 4 * 2 * HID], fp8, kind="ExternalInput").ap()
    wv8_in = nc.dram_tensor("wv8", [128, 4 * HID], fp8, kind="ExternalInput").ap()
    bwr_in = nc.dram_tensor("bwr", [128, 2 * HID], bf16, kind="ExternalInput").ap()
    bvr_in = nc.dram_tensor("bvr", [128, HID], bf16, kind="ExternalInput").ap()
    gam_in = nc.dram_tensor("gamma_b", [128, HID], f32, kind="ExternalInput").ap()
    bet_in = nc.dram_tensor("beta_b", [128, HID], f32, kind="ExternalInput").ap()
    ba_in = nc.dram_tensor("ba_b", [128, 1], f32, kind="ExternalInput").ap()
    wzf_in = nc.dram_tensor("wzf", [128, 64], fp8, kind="ExternalInput").ap()
    id2_in = nc.dram_tensor("id2", [128, 256], fp8, kind="ExternalInput").ap()
    xs_in = nc.dram_tensor("xs", [R, HID], f32, kind="ExternalInput").ap()
    gai_in = nc.dram_tensor("gai", [128, P // 16], mybir.dt.int16, kind="ExternalInput").ap()
    gbi_in = nc.dram_tensor("gbi", [128, P // 16], mybir.dt.int16, kind="ExternalInput").ap()
    oh8_in = nc.dram_tensor("oh8", [128, P], fp8, kind="ExternalInput").ap()
    y_out = nc.dram_tensor("y", [R, HID], f32, kind="ExternalOutput").ap()

    a_tbl = nc.dram_tensor("a_tbl", [R, 2 * HID], fp8, kind="Internal").ap()
    b_tbl = nc.dram_tensor("b_tbl", [N, 2 * HID], fp8, kind="Internal").ap()
    v_tbl = nc.dram_tensor("v_tbl", [N, HID], fp8, kind="Internal").ap()

    NT = N // 128
    G = B_pad // 128          # groups per block (even)

    with tile.TileContext(nc) as tc, ExitStack() as ctx:
        const = ctx.enter_context(tc.tile_pool(name="const", bufs=1))

        # f32 identity for PE transpose of p8
        iota_row = const.tile([128, 128], mybir.dt.int32)
        nc.gpsimd.iota(iota_row[:], pattern=[[1, 128]], base=0, channel_multiplier=0)
        pid = const.tile([128, 1], mybir.dt.int32)
        nc.gpsimd.iota(pid[:], pattern=[[0, 1]], base=0, channel_multiplier=1)
        iota_f = const.tile([128, 128], f32)
        nc.vector.tensor_copy(iota_f[:], iota_row[:])
        pid_f = const.tile([128, 1], f32)
        nc.vector.tensor_copy(pid_f[:], pid[:])
        ident = const.tile([128, 128], f32)
        nc.vector.tensor_scalar(ident[:], iota_f[:], pid_f[:], None, op0=Alu.is_equal)

        gam_sb = const.tile([128, HID], f32)
        nc.sync.dma_start(gam_sb[:], gam_in)
        bet_sb = const.tile([128, HID], f32)
        nc.sync.dma_start(bet_sb[:], bet_in)
        ba_sb = const.tile([128, 1], f32)
        nc.sync.dma_start(ba_sb[:], ba_in)
        wzf_sb = const.tile([128, 2, 32], fp8)
        nc.sync.dma_start(wzf_sb[:], wzf_in.rearrange("p (a b) -> p a b", a=2))
        id2_sb = const.tile([128, 2, 128], fp8)
        nc.sync.dma_start(id2_sb[:], id2_in.rearrange("p (a b) -> p a b", a=2))
        xs_sb = const.tile([128, BPC, HID], f32)
        nc.sync.dma_start(xs_sb[:], xs_in.rearrange("(b p) d -> p b d", p=128))
        gai_sb = const.tile([128, P // 16], mybir.dt.int16)
        nc.sync.dma_start(gai_sb[:], gai_in)
        gbi_sb = const.tile([128, P // 16], mybir.dt.int16)
        nc.sync.dma_start(gbi_sb[:], gbi_in)

        # ---------------- Stage 0: fp8 projection tables ----------------
        with ExitStack() as s0:
            wpool = s0.enter_context(tc.tile_pool(name="wpool", bufs=1))
            s0p = s0.enter_context(tc.tile_pool(name="s0p", bufs=3))
            psum_b = s0.enter_context(tc.tile_pool(name="psum_b", bufs=2, space="PSUM"))
            psum_a = s0.enter_context(tc.tile_pool(name="psum_a", bufs=1, space="PSUM"))
            psum_v = s0.enter_context(tc.tile_pool(name="psum_v", bufs=2, space="PSUM"))

            xt8_sb = wpool.tile([128, 4, N], fp8)
            nc.sync.dma_start(xt8_sb[:], xt8_in.rearrange("p (a n) -> p a n", a=4))
            xt8o_sb = wpool.tile([128, 4, R], fp8)
            nc.sync.dma_start(xt8o_sb[:], xt8o_in.rearrange("p (a n) -> p a n", a=4))
            wb8_sb = wpool.tile([128, 4, 2 * HID], fp8)
            nc.sync.dma_start(wb8_sb[:], wb8_in.rearrange("p (a n) -> p a n", a=4))
            wt8_sb = wpool.tile([128, 4, 2 * HID], fp8)
            nc.sync.dma_start(wt8_sb[:], wt8_in.rearrange("p (a n) -> p a n", a=4))
            wv8_sb = wpool.tile([128, 4, HID], fp8)
            nc.sync.dma_start(wv8_sb[:], wv8_in.rearrange("p (a n) -> p a n", a=4))
            bwr_sb = wpool.tile([128, 2 * HID], bf16)
            nc.sync.dma_start(bwr_sb[:], bwr_in)
            bvr_sb = wpool.tile([128, HID], bf16)
            nc.sync.dma_start(bvr_sb[:], bvr_in)
            ones1 = wpool.tile([128, 128], bf16)
            nc.vector.memset(ones1[:], 1.0)

            # a table first (tiny; gates ga gathers)
            a8 = s0p.tile([128, BPC, 2 * HID], fp8, tag="a8")
            for t in range(BPC):
                psa = psum_a.tile([128, 2 * HID], f32, tag="ps_a")
                for half in range(2):
                    hs = slice(half * HID, (half + 1) * HID)
                    nc.tensor.matmul(psa[:, hs], xt8o_sb[:, 0:2, t * 128:(t + 1) * 128],
                                     wt8_sb[:, 0:2, hs], start=True, stop=False, perf_mode=DR,
                                     skip_group_check=True)
                    nc.tensor.matmul(psa[:, hs], xt8o_sb[:, 2:4, t * 128:(t + 1) * 128],
                                     wt8_sb[:, 2:4, hs], start=False, stop=False, perf_mode=DR,
                                     skip_group_check=True)
                    nc.tensor.matmul(psa[:, hs], ones1[:], bwr_sb[:, hs], start=False,
                                     stop=True, skip_group_check=True)
                    if (t + half) % 2 == 0:
                        nc.vector.tensor_copy(a8[:, t, hs], psa[:, hs])
                    else:
                        nc.scalar.copy(a8[:, t, hs], psa[:, hs])
            nc.sync.dma_start(a_tbl.rearrange("(t p) f -> p t f", p=128), a8[:])
            # b table (gates gb gathers); write in 2-tile batches
            evac = 0
            for nt2 in range(NT // 2):
                b8 = s0p.tile([128, 2, 2 * HID], fp8, tag="b8")
                for k in range(2):
                    nt = 2 * nt2 + k
                    for half in range(2):
                        hs = slice(half * HID, (half + 1) * HID)
                        psb = psum_b.tile([128, HID], f32, tag="ps_b")
                        nc.tensor.matmul(psb[:], xt8_sb[:, 0:2, nt * 128:(nt + 1) * 128],
                                         wb8_sb[:, 0:2, hs], start=True, stop=False, perf_mode=DR)
                        nc.tensor.matmul(psb[:], xt8_sb[:, 2:4, nt * 128:(nt + 1) * 128],
                                         wb8_sb[:, 2:4, hs], start=False, stop=True, perf_mode=DR)
                        if evac % 4 < BEVAC_DVE_K:
                            nc.vector.tensor_copy(b8[:, k, hs], psb[:])
                        else:
                            nc.scalar.copy(b8[:, k, hs], psb[:])
                        evac += 1
                nc.sync.dma_start(
                    b_tbl[nt2 * 256:(nt2 + 1) * 256, :].rearrange("(t p) f -> p t f", p=128),
                    b8[:])
            # v table, 2-tile write batches
            for nt2 in range(NT // 2):
                v8 = s0p.tile([128, 2, HID], fp8, tag="v8")
                for k in range(2):
                    nt = 2 * nt2 + k
                    psv = psum_v.tile([128, HID], f32, tag="ps_v")
                    nc.tensor.matmul(psv[:], xt8_sb[:, 0:2, nt * 128:(nt + 1) * 128],
                                     wv8_sb[:, 0:2, :], start=True, stop=False, perf_mode=DR,
                                     skip_group_check=True)
                    nc.tensor.matmul(psv[:], xt8_sb[:, 2:4, nt * 128:(nt + 1) * 128],
                                     wv8_sb[:, 2:4, :], start=False, stop=False, perf_mode=DR,
                                     skip_group_check=True)
                    nc.tensor.matmul(psv[:], ones1[:], bvr_sb[:], start=False, stop=True,
                                     skip_group_check=True)
                    if nt % 2 == 0:
                        nc.vector.tensor_copy(v8[:, k, :], psv[:])
                    else:
                        nc.scalar.copy(v8[:, k, :], psv[:])
                nc.sync.dma_start(
                    v_tbl[nt2 * 256:(nt2 + 1) * 256, :].rearrange("(t p) f -> p t f", p=128),
                    v8[:])

        # ---------------- Edge stage ----------------
        s12 = ctx.enter_context(tc.tile_pool(name="s12", bufs=4))
        gvp = ctx.enter_context(tc.tile_pool(name="gvp", bufs=2))
        acc = ctx.enter_context(tc.tile_pool(name="acc", bufs=1, space="PSUM"))
        zqp = ctx.enter_context(tc.tile_pool(name="zqp", bufs=2, space="PSUM"))
        pswp = ctx.enter_context(tc.tile_pool(name="pswp", bufs=1, space="PSUM"))
        pstp = ctx.enter_context(tc.tile_pool(name="pstp", bufs=1, space="PSUM"))
        post = ctx.enter_context(tc.tile_pool(name="post", bufs=1))

        chunks = []
        off = 0
        while off < B_pad:
            c = min(512, B_pad - off)
            chunks.append((off, c))
            off += c

        lq = 0
        for blk in range(BPC):
            psum_y = acc.tile([128, HID], f32, tag="psum_y")
            psum_d = acc.tile([128, H], f32, tag="psum_d")
            base = blk * B_pad

            gv_blk = gvp.tile([128, G, HID], fp8, tag="gv")
            oh_blk = gvp.tile([128, G, 128], fp8, tag="ohb")
            nc.sync.dma_start(
                oh_blk[:], oh8_in[:, blk * G * 128:(blk + 1) * G * 128].rearrange(
                    "p (g r) -> p g r", g=G))

            def finish(st):
                p8, coff, C, NG, first, last = st
                pay = s12.tile([128, NG, HID + H], fp8, tag="pay")
                pst = pstp.tile([128, 4, H], f32, tag="pst")
                for g in range(NG):
                    nc.tensor.transpose(pst[:, g, :], p8[:, g * 128:(g + 1) * 128],
                                        ident[:H, :H])
                nc.vector.tensor_copy(pay[:, :, HID:], pst[:, :NG, :])
                nc.vector.tensor_mul(
                    pay[:, :, :HID].rearrange("p c (h d) -> p c h d", h=H),
                    gv_blk[:, coff // 128:coff // 128 + NG, :].rearrange(
                        "p c (h d) -> p c h d", h=H),
                    pay[:, :, HID:].unsqueeze(3).broadcast_to([128, NG, H, HD]))
                oh_c = oh_blk[:, coff // 128:coff // 128 + NG, :]
                for t in range(NG // 2):
                    st_ = first and t == 0
                    sp = last and t == NG // 2 - 1
                    nc.tensor.matmul(psum_y[:], oh_c[:, 2 * t:2 * t + 2, :],
                                     pay[:, 2 * t:2 * t + 2, :HID],
                                     start=st_, stop=sp, perf_mode=DR,
                                     skip_group_check=True)
                    nc.tensor.matmul(psum_d[:], oh_c[:, 2 * t:2 * t + 2, :],
                                     pay[:, 2 * t:2 * t + 2, HID:],
                                     start=st_, stop=sp, perf_mode=DR,
                                     skip_group_check=True)

            def dot_exp(st):
                zl2, coff, C, CH, NG, first, last = st
                psw = pswp.tile([8, 2, CH], f32, tag="psw")
                for eh in range(2):
                    for J in range(4):
                        nc.tensor.matmul(
                            psw[:, eh, :], wzf_sb[:, :, 8 * J:8 * J + 8],
                            zl2[:, 2 * J + eh, :, :],
                            start=(J == 0), stop=(J == 3), perf_mode=DR)
                p8 = s12.tile([8, C], f32, tag="p8")
                nc.scalar.activation(p8[:], psw[:].rearrange("p a c -> p (a c)"),
                                     Act.Exp, bias=ba_sb[:8, :], scale=1.0 / (WS * WAS))
                return (p8, coff, C, NG, first, last)

            prev_dot = None
            prev_fin = None
            for ci, (coff, C) in enumerate(chunks):
                off = base + coff
                i0, i1 = off // 16, (off + C) // 16
                CH = C // 2            # edges per eh-half / cc positions
                NG = C // 128          # groups in chunk

                gab = s12.tile([128, 2, 8, C], fp8, tag="gab")
                nc.gpsimd.dma_gather(
                    out_ap=gab[:, 0], in_ap=a_tbl, idxs_ap=gai_sb[:, i0:i1],
                    num_idxs=C, num_idxs_reg=C, elem_size=2 * HID, transpose=True)
                nc.gpsimd.dma_gather(
                    out_ap=gab[:, 1], in_ap=b_tbl, idxs_ap=gbi_sb[:, i0:i1],
                    num_idxs=C, num_idxs_reg=C, elem_size=2 * HID, transpose=True)
                nv = (B_pad + 1023) // 1024
                if ci < nv:
                    v0 = ci * 1024
                    vc = min(1024, B_pad - v0)
                    nc.gpsimd.dma_gather(
                        out_ap=gv_blk[:, v0 // 128:(v0 + vc) // 128, :], in_ap=v_tbl,
                        idxs_ap=gbi_sb[:, (base + v0) // 16:(base + v0 + vc) // 16],
                        num_idxs=vc, num_idxs_reg=vc, elem_size=HID)

                # z assembly + leaky -> zl2 [128, 8, 2, CH]
                zl2 = s12.tile([128, 8, 2, CH], fp8, tag="zl2")
                nhalf = C // 256
                for sg in range(2):
                    for ch in range(nhalf):
                        zq = zqp.tile([128, 4, 256], f32, tag="zq")
                        for jj in range(4):
                            j = 4 * sg + jj
                            nc.tensor.matmul(
                                zq[:, jj, :], id2_sb[:],
                                gab[:, :, j, ch * 256:(ch + 1) * 256],
                                start=True, stop=True, perf_mode=DR)
                        dst = zl2[:, 4 * sg:4 * sg + 4, :, ch * 128:(ch + 1) * 128]
                        dst = dst.rearrange("p s par cc -> p s cc par")
                        src = zq[:].rearrange("p s (cc par) -> p s cc par", par=2)
                        nc.scalar.activation(dst, src, Act.Prelu, alpha=0.01)

                if prev_dot is not None:
                    new_fin = dot_exp(prev_dot)
                    if prev_fin is not None:
                        finish(prev_fin)
                    prev_fin = new_fin
                prev_dot = (zl2, coff, C, CH, NG, ci == 0, ci == len(chunks) - 1)

            new_fin = dot_exp(prev_dot)
            if prev_fin is not None:
                finish(prev_fin)
            finish(new_fin)

            # ---------------- post: divide, ELU, residual, LayerNorm ----------------
            den = post.tile([128, H], f32, tag="den")
            nc.vector.tensor_scalar_add(den[:], psum_d[:], 1e-30)
            rden = post.tile([128, H], f32, tag="rden")
            nc.vector.reciprocal(rden[:], den[:])
            y1 = post.tile([128, HID], f32, tag="y1")
            nc.vector.tensor_mul(
                y1[:].rearrange("p (h d) -> p h d", h=H),
                psum_y[:].rearrange("p (h d) -> p h d", h=H),
                rden[:].unsqueeze(2).broadcast_to([128, H, HD]))
            m1 = post.tile([128, HID], f32, tag="m1")
            nc.vector.tensor_scalar_max(m1[:], y1[:], 0.0)
            t1 = post.tile([128, HID], f32, tag="t1")
            nc.vector.tensor_scalar_min(t1[:], y1[:], 0.0)
            t2 = post.tile([128, HID], f32, tag="t2")
            nc.scalar.activation(t2[:], t1[:], Act.Exp)
            y3 = post.tile([128, HID], f32, tag="y3")
            nc.vector.scalar_tensor_tensor(y3[:], t2[:], -1.0, m1[:],
                                           op0=Alu.add, op1=Alu.add)
            nc.vector.tensor_add(y3[:], y3[:], xs_sb[:, blk, :])
            mu = post.tile([128, 1], f32, tag="mu")
            nc.vector.reduce_sum(mu[:], y3[:], axis=mybir.AxisListType.X)
            nc.vector.tensor_scalar_mul(mu[:], mu[:], 1.0 / HID)
            yc = post.tile([128, HID], f32, tag="yc")
            nc.vector.tensor_scalar(yc[:], y3[:], mu[:], None, op0=Alu.subtract)
            sq = post.tile([128, HID], f32, tag="sq")
            nc.vector.tensor_mul(sq[:], yc[:], yc[:])
            s2 = post.tile([128, 1], f32, tag="s2")
            nc.vector.reduce_sum(s2[:], sq[:], axis=mybir.AxisListType.X)
            var = post.tile([128, 1], f32, tag="var")
            nc.vector.tensor_scalar(var[:], s2[:], 1.0 / HID, LN_EPS,
                                    op0=Alu.mult, op1=Alu.add)
            lnv = post.tile([128, 1], f32, tag="lnv")
            nc.scalar.activation(lnv[:], var[:], Act.Ln)
            rstd = post.tile([128, 1], f32, tag="rstd")
            nc.scalar.activation(rstd[:], lnv[:], Act.Exp, scale=-0.5)
            yn = post.tile([128, HID], f32, tag="yn")
            nc.vector.tensor_scalar(yn[:], yc[:], rstd[:], None, op0=Alu.mult)
            yf = post.tile([128, HID], f32, tag="yf")
            nc.vector.tensor_mul(yf[:], yn[:], gam_sb[:])
            nc.vector.tensor_add(yf[:], yf[:], bet_sb[:])
            nc.sync.dma_start(y_out[blk * 128:(blk + 1) * 128, :], yf[:])

    nc.compile()
    return nc


_CACHE = {}


def get_nc(B_pad, P):
    key = (B_pad, P)
    if key not in _CACHE:
        _CACHE[key] = build(B_pad, P)
    return _CACHE[key]


def kernel(**inputs) -> np.ndarray:
    in_maps, B_pad, P = prepare(**inputs)
    nc = get_nc(B_pad, P)
    res = run_bass_kernel_spmd(nc, in_maps, core_ids=list(range(NCORES)))
    out = np.concatenate([r["y"] for r in res.results], axis=0)
    return out.astype(np.float32)


if __name__ == "__main__":
    import reference
    inputs = {k: np.asarray(v) for k, v in reference.setup_inputs().items()}
    got = kernel(**inputs)
    want = np.asarray(reference.reference(**inputs))
    err = np.abs(got - want).max() / (np.abs(want).max() + 1e-12)
    print("abs-max relative error:", err)


# revision 28
# speedup vs baseline: 1.0023x; 1.0006x over previous
"""Multi-head GAT Bass kernel for 8 Trainium2 NeuronCores (fp8 edge pipeline).

Sharding: destination-node row-parallel (24 global blocks of 128 rows; core c
owns blocks 3c..3c+2 = 384 output rows). Edges bucketed by destination block
on the host, padded to a uniform per-block count (256-multiple). No
collectives; the host concatenates per-core outputs.

Per-core pipeline (all per-edge tensors fp8):
  stage0: b/a/v projection tables via fp8 DoubleRow matmuls (x, W in fp8,
          weights pre-scaled by 16 for fp8 range; psum f32, cast to fp8).
  edges:  transposed fp8 gathers of a[e0-local] and b[e1] into one tile;
          z = a+b via ONE DoubleRow matmul per slot (identity-pair k-tiles);
          leaky-relu on Act engine (PSUM -> fp8 SBUF) with a parity-
          deinterleaving output AP (the 16-bit-granularity transpose of the
          fp8 gather interleaves features at byte level);
          per-head dot with Wa via J-accumulated DoubleRow matmuls;
          exp on Act; PE transpose; payload p*v on DVE; onehot accumulation
          into psum via group-paired DoubleRow matmuls.
  post:   divide, ELU, residual, LayerNorm (as v1).
"""
import sys
sys.path.insert(0, '/opt/trn_rl_repo')

from contextlib import ExitStack

import numpy as np
import ml_dtypes

import concourse.bass as bass
import concourse.bacc as bacc
import concourse.tile as tile
from concourse import mybir
from concourse.bass_utils import run_bass_kernel_spmd

N = 3072
HID = 512
H = 8
HD = 64
E = 98304
LN_EPS = 1e-5
NCORES = 8
NBLK = 24
BPC = 3
R = 128 * BPC
WS = 16.0     # weight pre-scale for a/b tables (z stored x16)
WAS = 32.0    # wa pre-scale

f32 = mybir.dt.float32
bf16 = mybir.dt.bfloat16
fp8 = mybir.dt.float8e4
f8np = ml_dtypes.float8_e4m3fn
Alu = mybir.AluOpType
Act = mybir.ActivationFunctionType
DR = mybir.MatmulPerfMode.DoubleRow

# fraction of leaky-relu quarters on the DVE engine (rest on Act): k of 8
LEAKY_DVE_K = 0
# b-table evacuation: k of 4 halves on DVE (rest on Act)
BEVAC_DVE_K = 2


def _wrap_idx(idx):
    n = idx.shape[0]
    assert n % 16 == 0
    w16 = idx.reshape(n // 16, 16).T.astype(np.int16)
    return np.ascontiguousarray(np.tile(w16, (8, 1)))


def _ktile(a, k=4):
    """[K, M] -> [128, k, M] with row f = kk*128 + p."""
    K, M = a.shape
    assert K == 128 * k
    return np.ascontiguousarray(a.reshape(k, 128, M).transpose(1, 0, 2))


def prepare(x, edges, Wv, bv, Ww, bw, Wa, ba, gamma, beta):
    e0 = np.asarray(edges[0], np.int64) % N
    e1 = np.asarray(edges[1], np.int64) % N
    blk = e0 >> 7
    order = np.argsort(blk, kind="stable")
    counts = np.bincount(blk, minlength=NBLK)
    B_pad = max(256, int(-(-counts.max() // 256) * 256))
    P = BPC * B_pad

    ga_idx = np.zeros((NBLK, B_pad), np.int16)   # local (within-core) a rows
    gb_idx = np.zeros((NBLK, B_pad), np.int16)
    oh = np.zeros((NBLK, B_pad, 128), np.float32)
    starts = np.zeros(NBLK + 1, np.int64)
    starts[1:] = np.cumsum(counts)
    for b in range(NBLK):
        ids = order[starts[b]:starts[b + 1]]
        c = len(ids)
        core = b // BPC
        ga_idx[b, :c] = e0[ids] - 384 * core
        ga_idx[b, c:] = (b % BPC) * 128
        gb_idx[b, :c] = e1[ids]
        oh[b, np.arange(c), e0[ids] - b * 128] = 1.0

    x = np.asarray(x, np.float32)
    xT = np.ascontiguousarray(x.T)
    xt8 = _ktile(xT).astype(f8np)
    Ww = np.asarray(Ww, np.float32)
    wt8 = _ktile(Ww[:HID] * WS).astype(f8np)
    wb8 = _ktile(Ww[HID:] * WS).astype(f8np)
    wv8 = _ktile(np.asarray(Wv, np.float32)).astype(f8np)
    wa = np.asarray(Wa, np.float32).reshape(2 * HD)

    # wzf [128, 2, 32]: per J-slice [128, 2, 8]: col (2J+u) = wa[2*(p%64)+par]
    # for u == (p >= 64), else 0
    wzf = np.zeros((128, 2, 32), np.float32)
    pr = np.arange(128)
    for par in range(2):
        val = wa[2 * (pr % 64) + par] * WAS
        for J in range(4):
            u = (pr >= 64).astype(np.int64)
            wzf[pr, par, 8 * J + 2 * J + u] = val
    wzf8 = np.ascontiguousarray(wzf.astype(f8np).reshape(128, 64))

    id2 = np.zeros((128, 2, 128), np.float32)
    id2[pr, 0, pr] = 1.0
    id2[pr, 1, pr] = 1.0
    id2_8 = np.ascontiguousarray(id2.astype(f8np).reshape(128, 256))

    bwr = np.ascontiguousarray(np.broadcast_to(bw * WS / 128.0, (128, 2 * HID)).astype(ml_dtypes.bfloat16))
    bvr = np.ascontiguousarray(np.broadcast_to(np.asarray(bv, np.float32) / 128.0, (128, HID)).astype(ml_dtypes.bfloat16))
    gamma_b = np.ascontiguousarray(np.broadcast_to(gamma, (128, HID)).astype(np.float32))
    beta_b = np.ascontiguousarray(np.broadcast_to(beta, (128, HID)).astype(np.float32))
    ba_b = np.full((128, 1), float(np.asarray(ba).reshape(-1)[0]), np.float32)

    in_maps = []
    for c in range(NCORES):
        bs = slice(BPC * c, BPC * (c + 1))
        in_maps.append(dict(
            xt8=xt8,
            xt8o=np.ascontiguousarray(xt8[:, :, R * c:R * (c + 1)]),
            wb8=wb8, wt8=wt8, wv8=wv8,
            bwr=bwr, bvr=bvr,
            gamma_b=gamma_b, beta_b=beta_b, ba_b=ba_b,
            wzf=wzf8, id2=id2_8,
            xs=np.ascontiguousarray(x[R * c:R * (c + 1)]),
            gai=_wrap_idx(ga_idx[bs].reshape(-1)),
            gbi=_wrap_idx(gb_idx[bs].reshape(-1)),
            oh8=np.ascontiguousarray(
                oh[bs].reshape(BPC, B_pad // 128, 128, 128).transpose(2, 0, 1, 3)
                .reshape(128, BPC * (B_pad // 128) * 128).astype(f8np)),
        ))
    return in_maps, B_pad, P


def build(B_pad, P):
    nc = bacc.Bacc("TRN2", target_bir_lowering=False, num_devices=NCORES)

    xt8_in = nc.dram_tensor("xt8", [128, 4 * N], fp8, kind="ExternalInput").ap()
    xt8o_in = nc.dram_tensor("xt8o", [128, 4 * R], fp8, kind="ExternalInput").ap()
    wb8_in = nc.dram_tensor("wb8", [128, 4 * 2 * HID], fp8, kind="ExternalInput").ap()
    wt8_in = nc.dram_tensor("wt8", [128, 4 * 2 * HID], fp8, kind="ExternalInput").ap()
    wv8_in = nc.dram_tensor("wv8", [128, 4 * HID], fp8, kind="ExternalInput").ap()
    bwr_in = nc.dram_tensor("bwr", [128, 2 * HID], bf16, kind="ExternalInput").ap()
    bvr_in = nc.dram_tensor("bvr", [128, HID], bf16, kind="ExternalInput").ap()
    gam_in = nc.dram_tensor("gamma_b", [128, HID], f32, kind="ExternalInput").ap()
    bet_in = nc.dram_tensor("beta_b", [128, HID], f32, kind="ExternalInput").ap()
    ba_in = nc.dram_tensor("ba_b", [128, 1], f32, kind="ExternalInput").ap()
    wzf_in = nc.dram_tensor("wzf", [128, 64], fp8, kind="ExternalInput").ap()
    id2_in = nc.dram_tensor("id2", [128, 256], fp8, kind="ExternalInput").ap()
    xs_in = nc.dram_tensor("xs", [R, HID], f32, kind="ExternalInput").ap()
    gai_in = nc.dram_tensor("gai", [128, P // 16], mybir.dt.int16, kind="ExternalInput").ap()
    gbi_in = nc.dram_tensor("gbi", [128, P // 16], mybir.dt.int16, kind="ExternalInput").ap()
    oh8_in = nc.dram_tensor("oh8", [128, P], fp8, kind="ExternalInput").ap()
    y_out = nc.dram_tensor("y", [R, HID], f32, kind="ExternalOutput").ap()

    a_tbl = nc.dram_tensor("a_tbl", [R, 2 * HID], fp8, kind="Internal").ap()
    b_tbl = nc.dram_tensor("b_tbl", [N, 2 * HID], fp8, kind="Internal").ap()
    v_tbl = nc.dram_tensor("v_tbl", [N, HID], fp8, kind="Internal").ap()

    NT = N // 128
    G = B_pad // 128          # groups per block (even)

    with tile.TileContext(nc) as tc, ExitStack() as ctx:
        const = ctx.enter_context(tc.tile_pool(name="const", bufs=1))

        # f32 identity for PE transpose of p8
        iota_row = const.tile([128, 128], mybir.dt.int32)
        nc.gpsimd.iota(iota_row[:], pattern=[[1, 128]], base=0, channel_multiplier=0)
        pid = const.tile([128, 1], mybir.dt.int32)
        nc.gpsimd.iota(pid[:], pattern=[[0, 1]], base=0, channel_multiplier=1)
        iota_f = const.tile([128, 128], f32)
        nc.vector.tensor_copy(iota_f[:], iota_row[:])
        pid_f = const.tile([128, 1], f32)
        nc.vector.tensor_copy(pid_f[:], pid[:])
        ident = const.tile([128, 128], f32)
        nc.vector.tensor_scalar(ident[:], iota_f[:], pid_f[:], None, op0=Alu.is_equal)

        gam_sb = const.tile([128, HID], f32)
        nc.sync.dma_start(gam_sb[:], gam_in)
        bet_sb = const.tile([128, HID], f32)
        nc.sync.dma_start(bet_sb[:], bet_in)
        ba_sb = const.tile([128, 1], f32)
        nc.sync.dma_start(ba_sb[:], ba_in)
        wzf_sb = const.tile([128, 2, 32], fp8)
        nc.sync.dma_start(wzf_sb[:], wzf_in.rearrange("p (a b) -> p a b", a=2))
        id2_sb = const.tile([128, 2, 128], fp8)
        nc.sync.dma_start(id2_sb[:], id2_in.rearrange("p (a b) -> p a b", a=2))
        xs_sb = const.tile([128, BPC, HID], f32)
        nc.sync.dma_start(xs_sb[:], xs_in.rearrange("(b p) d -> p b d", p=128))
        gai_sb = const.tile([128, P // 16], mybir.dt.int16)
        nc.sync.dma_start(gai_sb[:], gai_in)
        gbi_sb = const.tile([128, P // 16], mybir.dt.int16)
        nc.sync.dma_start(gbi_sb[:], gbi_in)

        # ---------------- Stage 0: fp8 projection tables ----------------
        with ExitStack() as s0:
            wpool = s0.enter_context(tc.tile_pool(name="wpool", bufs=1))
            s0p = s0.enter_context(tc.tile_pool(name="s0p", bufs=3))
            psum_b = s0.enter_context(tc.tile_pool(name="psum_b", bufs=2, space="PSUM"))
            psum_a = s0.enter_context(tc.tile_pool(name="psum_a", bufs=1, space="PSUM"))
            psum_v = s0.enter_context(tc.tile_pool(name="psum_v", bufs=2, space="PSUM"))

            xt8_sb = wpool.tile([128, 4, N], fp8)
            nc.sync.dma_start(xt8_sb[:], xt8_in.rearrange("p (a n) -> p a n", a=4))
            xt8o_sb = wpool.tile([128, 4, R], fp8)
            nc.sync.dma_start(xt8o_sb[:], xt8o_in.rearrange("p (a n) -> p a n", a=4))
            wb8_sb = wpool.tile([128, 4, 2 * HID], fp8)
            nc.sync.dma_start(wb8_sb[:], wb8_in.rearrange("p (a n) -> p a n", a=4))
            wt8_sb = wpool.tile([128, 4, 2 * HID], fp8)
            nc.sync.dma_start(wt8_sb[:], wt8_in.rearrange("p (a n) -> p a n", a=4))
            wv8_sb = wpool.tile([128, 4, HID], fp8)
            nc.sync.dma_start(wv8_sb[:], wv8_in.rearrange("p (a n) -> p a n", a=4))
            bwr_sb = wpool.tile([128, 2 * HID], bf16)
            nc.sync.dma_start(bwr_sb[:], bwr_in)
            bvr_sb = wpool.tile([128, HID], bf16)
            nc.sync.dma_start(bvr_sb[:], bvr_in)
            ones1 = wpool.tile([128, 128], bf16)
            nc.vector.memset(ones1[:], 1.0)

            # a table first (tiny; gates ga gathers)
            a8 = s0p.tile([128, BPC, 2 * HID], fp8, tag="a8")
            for t in range(BPC):
                psa = psum_a.tile([128, 2 * HID], f32, tag="ps_a")
                for half in range(2):
                    hs = slice(half * HID, (half + 1) * HID)
                    nc.tensor.matmul(psa[:, hs], xt8o_sb[:, 0:2, t * 128:(t + 1) * 128],
                                     wt8_sb[:, 0:2, hs], start=True, stop=False, perf_mode=DR,
                                     skip_group_check=True)
                    nc.tensor.matmul(psa[:, hs], xt8o_sb[:, 2:4, t * 128:(t + 1) * 128],
                                     wt8_sb[:, 2:4, hs], start=False, stop=False, perf_mode=DR,
                                     skip_group_check=True)
                    nc.tensor.matmul(psa[:, hs], ones1[:], bwr_sb[:, hs], start=False,
                                     stop=True, skip_group_check=True)
                    if (t + half) % 2 == 0:
                        nc.vector.tensor_copy(a8[:, t, hs], psa[:, hs])
                    else:
                        nc.scalar.copy(a8[:, t, hs], psa[:, hs])
            nc.sync.dma_start(a_tbl.rearrange("(t p) f -> p t f", p=128), a8[:])
            # b table (gates gb gathers); write in 2-tile batches
            evac = 0
            for nt2 in range(NT // 2):
                b8 = s0p.tile([128, 2, 2 * HID], fp8, tag="b8")
                for k in range(2):
                    nt = 2 * nt2 + k
                    for half in range(2):
                        hs = slice(half * HID, (half + 1) * HID)
                        psb = psum_b.tile([128, HID], f32, tag="ps_b")
                        nc.tensor.matmul(psb[:], xt8_sb[:, 0:2, nt * 128:(nt + 1) * 128],
                                         wb8_sb[:, 0:2, hs], start=True, stop=False, perf_mode=DR)
                        nc.tensor.matmul(psb[:], xt8_sb[:, 2:4, nt * 128:(nt + 1) * 128],
                                         wb8_sb[:, 2:4, hs], start=False, stop=True, perf_mode=DR)
                        if evac % 4 < BEVAC_DVE_K:
                            nc.vector.tensor_copy(b8[:, k, hs], psb[:])
                        else:
                            nc.scalar.copy(b8[:, k, hs], psb[:])
                        evac += 1
                nc.sync.dma_start(
                    b_tbl[nt2 * 256:(nt2 + 1) * 256, :].rearrange("(t p) f -> p t f", p=128),
                    b8[:])
            # v table, 2-tile write batches
            for nt2 in range(NT // 2):
                v8 = s0p.tile([128, 2, HID], fp8, tag="v8")
                for k in range(2):
                    nt = 2 * nt2 + k
                    psv = psum_v.tile([128, HID], f32, tag="ps_v")
                    nc.tensor.matmul(psv[:], xt8_sb[:, 0:2, nt * 128:(nt + 1) * 128],
                                     wv8_sb[:, 0:2, :], start=True, stop=False, perf_mode=DR,
                                     skip_group_check=True)
                    nc.tensor.matmul(psv[:], xt8_sb[:, 2:4, nt * 128:(nt + 1) * 128],
                                     wv8_sb[:, 2:4, :], start=False, stop=False, perf_mode=DR,
                                     skip_group_check=True)
                    nc.tensor.matmul(psv[:], ones1[:], bvr_sb[:], start=False, stop=True,
                                     skip_group_check=True)
                    if nt % 2 == 0:
                        nc.vector.tensor_copy(v8[:, k, :], psv[:])
                    else:
                        nc.scalar.copy(v8[:, k, :], psv[:])
                nc.sync.dma_start(
                    v_tbl[nt2 * 256:(nt2 + 1) * 256, :].rearrange("(t p) f -> p t f", p=128),
                    v8[:])

        # ---------------- Edge stage ----------------
        s12 = ctx.enter_context(tc.tile_pool(name="s12", bufs=4))
        gvp = ctx.enter_context(tc.tile_pool(name="gvp", bufs=2))
        acc = ctx.enter_context(tc.tile_pool(name="acc", bufs=1, space="PSUM"))
        zqp = ctx.enter_context(tc.tile_pool(name="zqp", bufs=2, space="PSUM"))
        pswp = ctx.enter_context(tc.tile_pool(name="pswp", bufs=1, space="PSUM"))
        pstp = ctx.enter_context(tc.tile_pool(name="pstp", bufs=1, space="PSUM"))
        post = ctx.enter_context(tc.tile_pool(name="post", bufs=1))

        chunks = []
        off = 0
        while off < B_pad:
            c = min(512, B_pad - off)
            chunks.append((off, c))
            off += c

        lq = 0
        for blk in range(BPC):
            psum_y = acc.tile([128, HID], f32, tag="psum_y")
            psum_d = acc.tile([128, H], f32, tag="psum_d")
            base = blk * B_pad

            gv_blk = gvp.tile([128, G, HID], fp8, tag="gv")
            oh_blk = gvp.tile([128, G, 128], fp8, tag="ohb")
            nc.sync.dma_start(
                oh_blk[:], oh8_in[:, blk * G * 128:(blk + 1) * G * 128].rearrange(
                    "p (g r) -> p g r", g=G))

            def finish(st):
                p8, coff, C, NG, first, last = st
                pay = s12.tile([128, NG, HID + H], fp8, tag="pay")
                pst = pstp.tile([128, 4, H], f32, tag="pst")
                for g in range(NG):
                    nc.tensor.transpose(pst[:, g, :], p8[:, g * 128:(g + 1) * 128],
                                        ident[:H, :H])
                nc.vector.tensor_copy(pay[:, :, HID:], pst[:, :NG, :])
                nc.vector.tensor_mul(
                    pay[:, :, :HID].rearrange("p c (h d) -> p c h d", h=H),
                    gv_blk[:, coff // 128:coff // 128 + NG, :].rearrange(
                        "p c (h d) -> p c h d", h=H),
                    pay[:, :, HID:].unsqueeze(3).broadcast_to([128, NG, H, HD]))
                oh_c = oh_blk[:, coff // 128:coff // 128 + NG, :]
                for t in range(NG // 2):
                    st_ = first and t == 0
                    sp = last and t == NG // 2 - 1
                    nc.tensor.matmul(psum_y[:], oh_c[:, 2 * t:2 * t + 2, :],
                                     pay[:, 2 * t:2 * t + 2, :HID],
                                     start=st_, stop=sp, perf_mode=DR,
                                     skip_group_check=True)
                    nc.tensor.matmul(psum_d[:], oh_c[:, 2 * t:2 * t + 2, :],
                                     pay[:, 2 * t:2 * t + 2, HID:],
                                     start=st_, stop=sp, perf_mode=DR,
                                     skip_group_check=True)

            def exp_only(st):
                psw, coff, C, CH, NG, first, last = st
                p8 = s12.tile([8, C], f32, tag="p8")
                nc.scalar.activation(p8[:], psw[:].rearrange("p a c -> p (a c)"),
                                     Act.Exp, bias=ba_sb[:8, :], scale=1.0 / (WS * WAS))
                return (p8, coff, C, NG, first, last)

            prev_dot = None
            prev_fin = None
            for ci, (coff, C) in enumerate(chunks):
                off = base + coff
                i0, i1 = off // 16, (off + C) // 16
                CH = C // 2            # edges per eh-half / cc positions
                NG = C // 128          # groups in chunk

                gab = s12.tile([128, 2, 8, C], fp8, tag="gab")
                nc.gpsimd.dma_gather(
                    out_ap=gab[:, 0], in_ap=a_tbl, idxs_ap=gai_sb[:, i0:i1],
                    num_idxs=C, num_idxs_reg=C, elem_size=2 * HID, transpose=True)
                nc.gpsimd.dma_gather(
                    out_ap=gab[:, 1], in_ap=b_tbl, idxs_ap=gbi_sb[:, i0:i1],
                    num_idxs=C, num_idxs_reg=C, elem_size=2 * HID, transpose=True)
                nv = (B_pad + 1023) // 1024
                if ci < nv:
                    v0 = ci * 1024
                    vc = min(1024, B_pad - v0)
                    nc.gpsimd.dma_gather(
                        out_ap=gv_blk[:, v0 // 128:(v0 + vc) // 128, :], in_ap=v_tbl,
                        idxs_ap=gbi_sb[:, (base + v0) // 16:(base + v0 + vc) // 16],
                        num_idxs=vc, num_idxs_reg=vc, elem_size=HID)

                # z assembly + leaky -> zl2 [128, 8, 2, CH]; dot issued
                # incrementally (J 0-1 after slots 0-3, J 2-3 after 4-7)
                zl2 = s12.tile([128, 8, 2, CH], fp8, tag="zl2")
                psw = pswp.tile([8, 2, CH], f32, tag="psw")
                nhalf = C // 256
                for sg in range(2):
                    for ch in range(nhalf):
                        zq = zqp.tile([128, 4, 256], f32, tag="zq")
                        for jj in range(4):
                            j = 4 * sg + jj
                            nc.tensor.matmul(
                                zq[:, jj, :], id2_sb[:],
                                gab[:, :, j, ch * 256:(ch + 1) * 256],
                                start=True, stop=True, perf_mode=DR)
                        dst = zl2[:, 4 * sg:4 * sg + 4, :, ch * 128:(ch + 1) * 128]
                        dst = dst.rearrange("p s par cc -> p s cc par")
                        src = zq[:].rearrange("p s (cc par) -> p s cc par", par=2)
                        nc.scalar.activation(dst, src, Act.Prelu, alpha=0.01)
                    for eh in range(2):
                        for J in (0, 1) if sg == 0 else (2, 3):
                            nc.tensor.matmul(
                                psw[:, eh, :], wzf_sb[:, :, 8 * J:8 * J + 8],
                                zl2[:, 2 * J + eh, :, :],
                                start=(J == 0), stop=(J == 3), perf_mode=DR,
                                skip_group_check=True)

                if prev_dot is not None:
                    new_fin = exp_only(prev_dot)
                    if prev_fin is not None:
                        finish(prev_fin)
                    prev_fin = new_fin
                prev_dot = (psw, coff, C, CH, NG, ci == 0, ci == len(chunks) - 1)

            new_fin = exp_only(prev_dot)
            if prev_fin is not None:
                finish(prev_fin)
            finish(new_fin)

            # ---------------- post: divide, ELU, residual, LayerNorm ----------------
            den = post.tile([128, H], f32, tag="den")
            nc.vector.tensor_scalar_add(den[:], psum_d[:], 1e-30)
            rden = post.tile([128, H], f32, tag="rden")
            nc.vector.reciprocal(rden[:], den[:])
            y1 = post.tile([128, HID], f32, tag="y1")
            nc.vector.tensor_mul(
                y1[:].rearrange("p (h d) -> p h d", h=H),
                psum_y[:].rearrange("p (h d) -> p h d", h=H),
                rden[:].unsqueeze(2).broadcast_to([128, H, HD]))
            m1 = post.tile([128, HID], f32, tag="m1")
            nc.vector.tensor_scalar_max(m1[:], y1[:], 0.0)
            t1 = post.tile([128, HID], f32, tag="t1")
            nc.vector.tensor_scalar_min(t1[:], y1[:], 0.0)
            t2 = post.tile([128, HID], f32, tag="t2")
            nc.scalar.activation(t2[:], t1[:], Act.Exp)
            y3 = post.tile([128, HID], f32, tag="y3")
            nc.vector.scalar_tensor_tensor(y3[:], t2[:], -1.0, m1[:],
                                           op0=Alu.add, op1=Alu.add)
            nc.vector.tensor_add(y3[:], y3[:], xs_sb[:, blk, :])
            mu = post.tile([128, 1], f32, tag="mu")
            nc.vector.reduce_sum(mu[:], y3[:], axis=mybir.AxisListType.X)
            nc.vector.tensor_scalar_mul(mu[:], mu[:], 1.0 / HID)
            yc = post.tile([128, HID], f32, tag="yc")
            nc.vector.tensor_scalar(yc[:], y3[:], mu[:], None, op0=Alu.subtract)
            sq = post.tile([128, HID], f32, tag="sq")
            nc.vector.tensor_mul(sq[:], yc[:], yc[:])
            s2 = post.tile([128, 1], f32, tag="s2")
            nc.vector.reduce_sum(s2[:], sq[:], axis=mybir.AxisListType.X)
            var = post.tile([128, 1], f32, tag="var")
            nc.vector.tensor_scalar(var[:], s2[:], 1.0 / HID, LN_EPS,
                                    op0=Alu.mult, op1=Alu.add)
            lnv = post.tile([128, 1], f32, tag="lnv")
            nc.scalar.activation(lnv[:], var[:], Act.Ln)
            rstd = post.tile([128, 1], f32, tag="rstd")
            nc.scalar.activation(rstd[:], lnv[:], Act.Exp, scale=-0.5)
            yn = post.tile([128, HID], f32, tag="yn")
            nc.vector.tensor_scalar(yn[:], yc[:], rstd[:], None, op0=Alu.mult)
            yf = post.tile([128, HID], f32, tag="yf")
            nc.vector.tensor_mul(yf[:], yn[:], gam_sb[:])
            nc.vector.tensor_add(yf[:], yf[:], bet_sb[:])
            nc.sync.dma_start(y_out[blk * 128:(blk + 1) * 128, :], yf[:])

    nc.compile()
    return nc


_CACHE = {}


def get_nc(B_pad, P):
    key = (B_pad, P)
    if key not in _CACHE:
        _CACHE[key] = build(B_pad, P)
    return _CACHE[key]


def kernel(**inputs) -> np.ndarray:
    in_maps, B_pad, P = prepare(**inputs)
    nc = get_nc(B_pad, P)
    res = run_bass_kernel_spmd(nc, in_maps, core_ids=list(range(NCORES)))
    out = np.concatenate([r["y"] for r in res.results], axis=0)
    return out.astype(np.float32)


if __name__ == "__main__":
    import reference
    inputs = {k: np.asarray(v) for k, v in reference.setup_inputs().items()}
    got = kernel(**inputs)
    want = np.asarray(reference.reference(**inputs))
    err = np.abs(got - want).max() / (np.abs(want).max() + 1e-12)
    print("abs-max relative error:", err)


# revision 29
# speedup vs baseline: 1.0326x; 1.0303x over previous
"""Multi-head GAT Bass kernel for 8 Trainium2 NeuronCores (fp8 edge pipeline).

Sharding: destination-node row-parallel (24 global blocks of 128 rows; core c
owns blocks 3c..3c+2 = 384 output rows). Edges bucketed by destination block
on the host, padded to a uniform per-block count (256-multiple). No
collectives; the host concatenates per-core outputs.

Per-core pipeline (all per-edge tensors fp8):
  stage0: b/a/v projection tables via fp8 DoubleRow matmuls (x, W in fp8,
          weights pre-scaled by 16 for fp8 range; psum f32, cast to fp8).
  edges:  transposed fp8 gathers of a[e0-local] and b[e1] into one tile;
          z = a+b via ONE DoubleRow matmul per slot (identity-pair k-tiles);
          leaky-relu on Act engine (PSUM -> fp8 SBUF) with a parity-
          deinterleaving output AP (the 16-bit-granularity transpose of the
          fp8 gather interleaves features at byte level);
          per-head dot with Wa via J-accumulated DoubleRow matmuls;
          exp on Act; PE transpose; payload p*v on DVE; onehot accumulation
          into psum via group-paired DoubleRow matmuls.
  post:   divide, ELU, residual, LayerNorm (as v1).
"""
import sys
sys.path.insert(0, '/opt/trn_rl_repo')

from contextlib import ExitStack

import numpy as np
import ml_dtypes

import concourse.bass as bass
import concourse.bacc as bacc
import concourse.tile as tile
from concourse import mybir
from concourse.bass_utils import run_bass_kernel_spmd

N = 3072
HID = 512
H = 8
HD = 64
E = 98304
LN_EPS = 1e-5
NCORES = 8
NBLK = 24
BPC = 3
R = 128 * BPC
WS = 16.0     # weight pre-scale for a/b tables (z stored x16)
WAS = 32.0    # wa pre-scale

f32 = mybir.dt.float32
bf16 = mybir.dt.bfloat16
fp8 = mybir.dt.float8e4
f8np = ml_dtypes.float8_e4m3fn
Alu = mybir.AluOpType
Act = mybir.ActivationFunctionType
DR = mybir.MatmulPerfMode.DoubleRow

# fraction of leaky-relu quarters on the DVE engine (rest on Act): k of 8
LEAKY_DVE_K = 0
# b-table evacuation: k of 4 halves on DVE (rest on Act)
BEVAC_DVE_K = 2


def _wrap_idx(idx):
    n = idx.shape[0]
    assert n % 16 == 0
    w16 = idx.reshape(n // 16, 16).T.astype(np.int16)
    return np.ascontiguousarray(np.tile(w16, (8, 1)))


def _ktile(a, k=4):
    """[K, M] -> [128, k, M] with row f = kk*128 + p."""
    K, M = a.shape
    assert K == 128 * k
    return np.ascontiguousarray(a.reshape(k, 128, M).transpose(1, 0, 2))


def prepare(x, edges, Wv, bv, Ww, bw, Wa, ba, gamma, beta):
    e0 = np.asarray(edges[0], np.int64) % N
    e1 = np.asarray(edges[1], np.int64) % N
    blk = e0 >> 7
    order = np.argsort(blk, kind="stable")
    counts = np.bincount(blk, minlength=NBLK)
    B_pad = max(256, int(-(-counts.max() // 256) * 256))
    P = BPC * B_pad

    ga_idx = np.zeros((NBLK, B_pad), np.int16)   # local (within-core) a rows
    gb_idx = np.zeros((NBLK, B_pad), np.int16)
    oh = np.zeros((NBLK, B_pad, 128), np.float32)
    starts = np.zeros(NBLK + 1, np.int64)
    starts[1:] = np.cumsum(counts)
    for b in range(NBLK):
        ids = order[starts[b]:starts[b + 1]]
        c = len(ids)
        core = b // BPC
        ga_idx[b, :c] = e0[ids] - 384 * core
        ga_idx[b, c:] = (b % BPC) * 128
        gb_idx[b, :c] = e1[ids]
        oh[b, np.arange(c), e0[ids] - b * 128] = 1.0

    x = np.asarray(x, np.float32)
    xT = np.ascontiguousarray(x.T)
    xt8 = _ktile(xT).astype(f8np)
    Ww = np.asarray(Ww, np.float32)
    wt8 = _ktile(Ww[:HID] * WS).astype(f8np)
    wb8 = _ktile(Ww[HID:] * WS).astype(f8np)
    wv8 = _ktile(np.asarray(Wv, np.float32)).astype(f8np)
    wa = np.asarray(Wa, np.float32).reshape(2 * HD)

    # wzf [128, 2, 32]: per J-slice [128, 2, 8]: col (2J+u) = wa[2*(p%64)+par]
    # for u == (p >= 64), else 0
    wzf = np.zeros((128, 2, 32), np.float32)
    pr = np.arange(128)
    for par in range(2):
        val = wa[2 * (pr % 64) + par] * WAS
        for J in range(4):
            u = (pr >= 64).astype(np.int64)
            wzf[pr, par, 8 * J + 2 * J + u] = val
    wzf8 = np.ascontiguousarray(wzf.astype(f8np).reshape(128, 64))

    id2 = np.zeros((128, 2, 128), np.float32)
    id2[pr, 0, pr] = 1.0
    id2[pr, 1, pr] = 1.0
    id2_8 = np.ascontiguousarray(id2.astype(f8np).reshape(128, 256))

    bwr = np.ascontiguousarray(np.broadcast_to(bw * WS / 128.0, (128, 2 * HID)).astype(ml_dtypes.bfloat16))
    bvr = np.ascontiguousarray(np.broadcast_to(np.asarray(bv, np.float32) / 128.0, (128, HID)).astype(ml_dtypes.bfloat16))
    gamma_b = np.ascontiguousarray(np.broadcast_to(gamma, (128, HID)).astype(np.float32))
    beta_b = np.ascontiguousarray(np.broadcast_to(beta, (128, HID)).astype(np.float32))
    ba_b = np.full((128, 1), float(np.asarray(ba).reshape(-1)[0]), np.float32)

    in_maps = []
    for c in range(NCORES):
        bs = slice(BPC * c, BPC * (c + 1))
        in_maps.append(dict(
            xt8=xt8,
            xt8o=np.ascontiguousarray(xt8[:, :, R * c:R * (c + 1)]),
            wb8=wb8, wt8=wt8, wv8=wv8,
            bwr=bwr, bvr=bvr,
            gamma_b=gamma_b, beta_b=beta_b, ba_b=ba_b,
            wzf=wzf8, id2=id2_8,
            xs=np.ascontiguousarray(x[R * c:R * (c + 1)]),
            gai=_wrap_idx(ga_idx[bs].reshape(-1)),
            gbi=_wrap_idx(gb_idx[bs].reshape(-1)),
            oh8=np.ascontiguousarray(
                oh[bs].reshape(BPC, B_pad // 128, 128, 128).transpose(2, 0, 1, 3)
                .reshape(128, BPC * (B_pad // 128) * 128).astype(f8np)),
        ))
    return in_maps, B_pad, P


def build(B_pad, P):
    nc = bacc.Bacc("TRN2", target_bir_lowering=False, num_devices=NCORES)

    xt8_in = nc.dram_tensor("xt8", [128, 4 * N], fp8, kind="ExternalInput").ap()
    xt8o_in = nc.dram_tensor("xt8o", [128, 4 * R], fp8, kind="ExternalInput").ap()
    wb8_in = nc.dram_tensor("wb8", [128, 4 * 2 * HID], fp8, kind="ExternalInput").ap()
    wt8_in = nc.dram_tensor("wt8", [128, 4 * 2 * HID], fp8, kind="ExternalInput").ap()
    wv8_in = nc.dram_tensor("wv8", [128, 4 * HID], fp8, kind="ExternalInput").ap()
    bwr_in = nc.dram_tensor("bwr", [128, 2 * HID], bf16, kind="ExternalInput").ap()
    bvr_in = nc.dram_tensor("bvr", [128, HID], bf16, kind="ExternalInput").ap()
    gam_in = nc.dram_tensor("gamma_b", [128, HID], f32, kind="ExternalInput").ap()
    bet_in = nc.dram_tensor("beta_b", [128, HID], f32, kind="ExternalInput").ap()
    ba_in = nc.dram_tensor("ba_b", [128, 1], f32, kind="ExternalInput").ap()
    wzf_in = nc.dram_tensor("wzf", [128, 64], fp8, kind="ExternalInput").ap()
    id2_in = nc.dram_tensor("id2", [128, 256], fp8, kind="ExternalInput").ap()
    xs_in = nc.dram_tensor("xs", [R, HID], f32, kind="ExternalInput").ap()
    gai_in = nc.dram_tensor("gai", [128, P // 16], mybir.dt.int16, kind="ExternalInput").ap()
    gbi_in = nc.dram_tensor("gbi", [128, P // 16], mybir.dt.int16, kind="ExternalInput").ap()
    oh8_in = nc.dram_tensor("oh8", [128, P], fp8, kind="ExternalInput").ap()
    y_out = nc.dram_tensor("y", [R, HID], f32, kind="ExternalOutput").ap()

    a_tbl = nc.dram_tensor("a_tbl", [R, 2 * HID], fp8, kind="Internal").ap()
    b_tbl = nc.dram_tensor("b_tbl", [N, 2 * HID], fp8, kind="Internal").ap()
    v_tbl = nc.dram_tensor("v_tbl", [N, HID], fp8, kind="Internal").ap()

    NT = N // 128
    G = B_pad // 128          # groups per block (even)

    with tile.TileContext(nc, pool_alloc_mode="queue") as tc, ExitStack() as ctx:
        const = ctx.enter_context(tc.tile_pool(name="const", bufs=1))

        # f32 identity for PE transpose of p8
        iota_row = const.tile([128, 128], mybir.dt.int32)
        nc.gpsimd.iota(iota_row[:], pattern=[[1, 128]], base=0, channel_multiplier=0)
        pid = const.tile([128, 1], mybir.dt.int32)
        nc.gpsimd.iota(pid[:], pattern=[[0, 1]], base=0, channel_multiplier=1)
        iota_f = const.tile([128, 128], f32)
        nc.vector.tensor_copy(iota_f[:], iota_row[:])
        pid_f = const.tile([128, 1], f32)
        nc.vector.tensor_copy(pid_f[:], pid[:])
        ident = const.tile([128, 128], f32)
        nc.vector.tensor_scalar(ident[:], iota_f[:], pid_f[:], None, op0=Alu.is_equal)

        gam_sb = const.tile([128, HID], f32)
        nc.sync.dma_start(gam_sb[:], gam_in)
        bet_sb = const.tile([128, HID], f32)
        nc.sync.dma_start(bet_sb[:], bet_in)
        ba_sb = const.tile([128, 1], f32)
        nc.sync.dma_start(ba_sb[:], ba_in)
        wzf_sb = const.tile([128, 2, 32], fp8)
        nc.sync.dma_start(wzf_sb[:], wzf_in.rearrange("p (a b) -> p a b", a=2))
        id2_sb = const.tile([128, 2, 128], fp8)
        nc.sync.dma_start(id2_sb[:], id2_in.rearrange("p (a b) -> p a b", a=2))
        xs_sb = const.tile([128, BPC, HID], f32)
        nc.sync.dma_start(xs_sb[:], xs_in.rearrange("(b p) d -> p b d", p=128))
        gai_sb = const.tile([128, P // 16], mybir.dt.int16)
        nc.sync.dma_start(gai_sb[:], gai_in)
        gbi_sb = const.tile([128, P // 16], mybir.dt.int16)
        nc.sync.dma_start(gbi_sb[:], gbi_in)

        # ---------------- Stage 0: fp8 projection tables ----------------
        with ExitStack() as s0:
            wpool = s0.enter_context(tc.tile_pool(name="wpool", bufs=1))
            s0p = s0.enter_context(tc.tile_pool(name="s0p", bufs=3))
            psum_b = s0.enter_context(tc.tile_pool(name="psum_b", bufs=2, space="PSUM"))
            psum_a = s0.enter_context(tc.tile_pool(name="psum_a", bufs=1, space="PSUM"))
            psum_v = s0.enter_context(tc.tile_pool(name="psum_v", bufs=2, space="PSUM"))

            xt8_sb = wpool.tile([128, 4, N], fp8)
            nc.sync.dma_start(xt8_sb[:], xt8_in.rearrange("p (a n) -> p a n", a=4))
            xt8o_sb = wpool.tile([128, 4, R], fp8)
            nc.sync.dma_start(xt8o_sb[:], xt8o_in.rearrange("p (a n) -> p a n", a=4))
            wb8_sb = wpool.tile([128, 4, 2 * HID], fp8)
            nc.sync.dma_start(wb8_sb[:], wb8_in.rearrange("p (a n) -> p a n", a=4))
            wt8_sb = wpool.tile([128, 4, 2 * HID], fp8)
            nc.sync.dma_start(wt8_sb[:], wt8_in.rearrange("p (a n) -> p a n", a=4))
            wv8_sb = wpool.tile([128, 4, HID], fp8)
            nc.sync.dma_start(wv8_sb[:], wv8_in.rearrange("p (a n) -> p a n", a=4))
            bwr_sb = wpool.tile([128, 2 * HID], bf16)
            nc.sync.dma_start(bwr_sb[:], bwr_in)
            bvr_sb = wpool.tile([128, HID], bf16)
            nc.sync.dma_start(bvr_sb[:], bvr_in)
            ones1 = wpool.tile([128, 128], bf16)
            nc.vector.memset(ones1[:], 1.0)

            # a table first (tiny; gates ga gathers)
            a8 = s0p.tile([128, BPC, 2 * HID], fp8, tag="a8")
            for t in range(BPC):
                psa = psum_a.tile([128, 2 * HID], f32, tag="ps_a")
                for half in range(2):
                    hs = slice(half * HID, (half + 1) * HID)
                    nc.tensor.matmul(psa[:, hs], xt8o_sb[:, 0:2, t * 128:(t + 1) * 128],
                                     wt8_sb[:, 0:2, hs], start=True, stop=False, perf_mode=DR,
                                     skip_group_check=True)
                    nc.tensor.matmul(psa[:, hs], xt8o_sb[:, 2:4, t * 128:(t + 1) * 128],
                                     wt8_sb[:, 2:4, hs], start=False, stop=False, perf_mode=DR,
                                     skip_group_check=True)
                    nc.tensor.matmul(psa[:, hs], ones1[:], bwr_sb[:, hs], start=False,
                                     stop=True, skip_group_check=True)
                    if (t + half) % 2 == 0:
                        nc.vector.tensor_copy(a8[:, t, hs], psa[:, hs])
                    else:
                        nc.scalar.copy(a8[:, t, hs], psa[:, hs])
            nc.sync.dma_start(a_tbl.rearrange("(t p) f -> p t f", p=128), a8[:])
            # b table (gates gb gathers); write in 2-tile batches
            evac = 0
            for nt2 in range(NT // 2):
                b8 = s0p.tile([128, 2, 2 * HID], fp8, tag="b8")
                for k in range(2):
                    nt = 2 * nt2 + k
                    for half in range(2):
                        hs = slice(half * HID, (half + 1) * HID)
                        psb = psum_b.tile([128, HID], f32, tag="ps_b")
                        nc.tensor.matmul(psb[:], xt8_sb[:, 0:2, nt * 128:(nt + 1) * 128],
                                         wb8_sb[:, 0:2, hs], start=True, stop=False, perf_mode=DR)
                        nc.tensor.matmul(psb[:], xt8_sb[:, 2:4, nt * 128:(nt + 1) * 128],
                                         wb8_sb[:, 2:4, hs], start=False, stop=True, perf_mode=DR)
                        if evac % 4 < BEVAC_DVE_K:
                            nc.vector.tensor_copy(b8[:, k, hs], psb[:])
                        else:
                            nc.scalar.copy(b8[:, k, hs], psb[:])
                        evac += 1
                nc.sync.dma_start(
                    b_tbl[nt2 * 256:(nt2 + 1) * 256, :].rearrange("(t p) f -> p t f", p=128),
                    b8[:])
            # v table, 2-tile write batches
            for nt2 in range(NT // 2):
                v8 = s0p.tile([128, 2, HID], fp8, tag="v8")
                for k in range(2):
                    nt = 2 * nt2 + k
                    psv = psum_v.tile([128, HID], f32, tag="ps_v")
                    nc.tensor.matmul(psv[:], xt8_sb[:, 0:2, nt * 128:(nt + 1) * 128],
                                     wv8_sb[:, 0:2, :], start=True, stop=False, perf_mode=DR,
                                     skip_group_check=True)
                    nc.tensor.matmul(psv[:], xt8_sb[:, 2:4, nt * 128:(nt + 1) * 128],
                                     wv8_sb[:, 2:4, :], start=False, stop=False, perf_mode=DR,
                                     skip_group_check=True)
                    nc.tensor.matmul(psv[:], ones1[:], bvr_sb[:], start=False, stop=True,
                                     skip_group_check=True)
                    if nt % 2 == 0:
                        nc.vector.tensor_copy(v8[:, k, :], psv[:])
                    else:
                        nc.scalar.copy(v8[:, k, :], psv[:])
                nc.sync.dma_start(
                    v_tbl[nt2 * 256:(nt2 + 1) * 256, :].rearrange("(t p) f -> p t f", p=128),
                    v8[:])

        # ---------------- Edge stage ----------------
        s12 = ctx.enter_context(tc.tile_pool(name="s12", bufs=4))
        gvp = ctx.enter_context(tc.tile_pool(name="gvp", bufs=2))
        acc = ctx.enter_context(tc.tile_pool(name="acc", bufs=1, space="PSUM"))
        zqp = ctx.enter_context(tc.tile_pool(name="zqp", bufs=2, space="PSUM"))
        pswp = ctx.enter_context(tc.tile_pool(name="pswp", bufs=1, space="PSUM"))
        pstp = ctx.enter_context(tc.tile_pool(name="pstp", bufs=1, space="PSUM"))
        post = ctx.enter_context(tc.tile_pool(name="post", bufs=1))

        chunks = []
        off = 0
        while off < B_pad:
            c = min(512, B_pad - off)
            chunks.append((off, c))
            off += c

        lq = 0
        for blk in range(BPC):
            psum_y = acc.tile([128, HID], f32, tag="psum_y")
            psum_d = acc.tile([128, H], f32, tag="psum_d")
            base = blk * B_pad

            gv_blk = gvp.tile([128, G, HID], fp8, tag="gv")
            oh_blk = gvp.tile([128, G, 128], fp8, tag="ohb")
            nc.sync.dma_start(
                oh_blk[:], oh8_in[:, blk * G * 128:(blk + 1) * G * 128].rearrange(
                    "p (g r) -> p g r", g=G))

            def finish(st):
                p8, coff, C, NG, first, last = st
                pay = s12.tile([128, NG, HID + H], fp8, tag="pay")
                pst = pstp.tile([128, 4, H], f32, tag="pst")
                for g in range(NG):
                    nc.tensor.transpose(pst[:, g, :], p8[:, g * 128:(g + 1) * 128],
                                        ident[:H, :H])
                nc.vector.tensor_copy(pay[:, :, HID:], pst[:, :NG, :])
                nc.vector.tensor_mul(
                    pay[:, :, :HID].rearrange("p c (h d) -> p c h d", h=H),
                    gv_blk[:, coff // 128:coff // 128 + NG, :].rearrange(
                        "p c (h d) -> p c h d", h=H),
                    pay[:, :, HID:].unsqueeze(3).broadcast_to([128, NG, H, HD]))
                oh_c = oh_blk[:, coff // 128:coff // 128 + NG, :]
                for t in range(NG // 2):
                    st_ = first and t == 0
                    sp = last and t == NG // 2 - 1
                    nc.tensor.matmul(psum_y[:], oh_c[:, 2 * t:2 * t + 2, :],
                                     pay[:, 2 * t:2 * t + 2, :HID],
                                     start=st_, stop=sp, perf_mode=DR,
                                     skip_group_check=True)
                    nc.tensor.matmul(psum_d[:], oh_c[:, 2 * t:2 * t + 2, :],
                                     pay[:, 2 * t:2 * t + 2, HID:],
                                     start=st_, stop=sp, perf_mode=DR,
                                     skip_group_check=True)

            def exp_only(st):
                psw, coff, C, CH, NG, first, last = st
                p8 = s12.tile([8, C], f32, tag="p8")
                nc.scalar.activation(p8[:], psw[:].rearrange("p a c -> p (a c)"),
                                     Act.Exp, bias=ba_sb[:8, :], scale=1.0 / (WS * WAS))
                return (p8, coff, C, NG, first, last)

            prev_dot = None
            prev_fin = None
            for ci, (coff, C) in enumerate(chunks):
                off = base + coff
                i0, i1 = off // 16, (off + C) // 16
                CH = C // 2            # edges per eh-half / cc positions
                NG = C // 128          # groups in chunk

                gab = s12.tile([128, 2, 8, C], fp8, tag="gab")
                nc.gpsimd.dma_gather(
                    out_ap=gab[:, 0], in_ap=a_tbl, idxs_ap=gai_sb[:, i0:i1],
                    num_idxs=C, num_idxs_reg=C, elem_size=2 * HID, transpose=True)
                nc.gpsimd.dma_gather(
                    out_ap=gab[:, 1], in_ap=b_tbl, idxs_ap=gbi_sb[:, i0:i1],
                    num_idxs=C, num_idxs_reg=C, elem_size=2 * HID, transpose=True)
                nv = (B_pad + 1023) // 1024
                if ci < nv:
                    v0 = ci * 1024
                    vc = min(1024, B_pad - v0)
                    nc.gpsimd.dma_gather(
                        out_ap=gv_blk[:, v0 // 128:(v0 + vc) // 128, :], in_ap=v_tbl,
                        idxs_ap=gbi_sb[:, (base + v0) // 16:(base + v0 + vc) // 16],
                        num_idxs=vc, num_idxs_reg=vc, elem_size=HID)

                # z assembly + leaky -> zl2 [128, 8, 2, CH]; dot issued
                # incrementally (J 0-1 after slots 0-3, J 2-3 after 4-7)
                zl2 = s12.tile([128, 8, 2, CH], fp8, tag="zl2")
                psw = pswp.tile([8, 2, CH], f32, tag="psw")
                nhalf = C // 256
                for sg in range(2):
                    for ch in range(nhalf):
                        zq = zqp.tile([128, 4, 256], f32, tag="zq")
                        for jj in range(4):
                            j = 4 * sg + jj
                            nc.tensor.matmul(
                                zq[:, jj, :], id2_sb[:],
                                gab[:, :, j, ch * 256:(ch + 1) * 256],
                                start=True, stop=True, perf_mode=DR)
                        dst = zl2[:, 4 * sg:4 * sg + 4, :, ch * 128:(ch + 1) * 128]
                        dst = dst.rearrange("p s par cc -> p s cc par")
                        src = zq[:].rearrange("p s (cc par) -> p s cc par", par=2)
                        nc.scalar.activation(dst, src, Act.Prelu, alpha=0.01)
                    for eh in range(2):
                        for J in (0, 1) if sg == 0 else (2, 3):
                            nc.tensor.matmul(
                                psw[:, eh, :], wzf_sb[:, :, 8 * J:8 * J + 8],
                                zl2[:, 2 * J + eh, :, :],
                                start=(J == 0), stop=(J == 3), perf_mode=DR,
                                skip_group_check=True)

                if prev_dot is not None:
                    new_fin = exp_only(prev_dot)
                    if prev_fin is not None:
                        finish(prev_fin)
                    prev_fin = new_fin
                prev_dot = (psw, coff, C, CH, NG, ci == 0, ci == len(chunks) - 1)

            new_fin = exp_only(prev_dot)
            if prev_fin is not None:
                finish(prev_fin)
            finish(new_fin)

            # ---------------- post: divide, ELU, residual, LayerNorm ----------------
            den = post.tile([128, H], f32, tag="den")
            nc.vector.tensor_scalar_add(den[:], psum_d[:], 1e-30)
            rden = post.tile([128, H], f32, tag="rden")
            nc.vector.reciprocal(rden[:], den[:])
            y1 = post.tile([128, HID], f32, tag="y1")
            nc.vector.tensor_mul(
                y1[:].rearrange("p (h d) -> p h d", h=H),
                psum_y[:].rearrange("p (h d) -> p h d", h=H),
                rden[:].unsqueeze(2).broadcast_to([128, H, HD]))
            m1 = post.tile([128, HID], f32, tag="m1")
            nc.vector.tensor_scalar_max(m1[:], y1[:], 0.0)
            t1 = post.tile([128, HID], f32, tag="t1")
            nc.vector.tensor_scalar_min(t1[:], y1[:], 0.0)
            t2 = post.tile([128, HID], f32, tag="t2")
            nc.scalar.activation(t2[:], t1[:], Act.Exp)
            y3 = post.tile([128, HID], f32, tag="y3")
            nc.vector.scalar_tensor_tensor(y3[:], t2[:], -1.0, m1[:],
                                           op0=Alu.add, op1=Alu.add)
            nc.vector.tensor_add(y3[:], y3[:], xs_sb[:, blk, :])
            mu = post.tile([128, 1], f32, tag="mu")
            nc.vector.reduce_sum(mu[:], y3[:], axis=mybir.AxisListType.X)
            nc.vector.tensor_scalar_mul(mu[:], mu[:], 1.0 / HID)
            yc = post.tile([128, HID], f32, tag="yc")
            nc.vector.tensor_scalar(yc[:], y3[:], mu[:], None, op0=Alu.subtract)
            sq = post.tile([128, HID], f32, tag="sq")
            nc.vector.tensor_mul(sq[:], yc[:], yc[:])
            s2 = post.tile([128, 1], f32, tag="s2")
            nc.vector.reduce_sum(s2[:], sq[:], axis=mybir.AxisListType.X)
            var = post.tile([128, 1], f32, tag="var")
            nc.vector.tensor_scalar(var[:], s2[:], 1.0 / HID, LN_EPS,
                                    op0=Alu.mult, op1=Alu.add)
            lnv = post.tile([128, 1], f32, tag="lnv")
            nc.scalar.activation(lnv[:], var[:], Act.Ln)
            rstd = post.tile([128, 1], f32, tag="rstd")
            nc.scalar.activation(rstd[:], lnv[:], Act.Exp, scale=-0.5)
            yn = post.tile([128, HID], f32, tag="yn")
            nc.vector.tensor_scalar(yn[:], yc[:], rstd[:], None, op0=Alu.mult)
            yf = post.tile([128, HID], f32, tag="yf")
            nc.vector.tensor_mul(yf[:], yn[:], gam_sb[:])
            nc.vector.tensor_add(yf[:], yf[:], bet_sb[:])
            nc.sync.dma_start(y_out[blk * 128:(blk + 1) * 128, :], yf[:])

    nc.compile()
    return nc


_CACHE = {}


def get_nc(B_pad, P):
    key = (B_pad, P)
    if key not in _CACHE:
        _CACHE[key] = build(B_pad, P)
    return _CACHE[key]


def kernel(**inputs) -> np.ndarray:
    in_maps, B_pad, P = prepare(**inputs)
    nc = get_nc(B_pad, P)
    res = run_bass_kernel_spmd(nc, in_maps, core_ids=list(range(NCORES)))
    out = np.concatenate([r["y"] for r in res.results], axis=0)
    return out.astype(np.float32)


if __name__ == "__main__":
    import reference
    inputs = {k: np.asarray(v) for k, v in reference.setup_inputs().items()}
    got = kernel(**inputs)
    want = np.asarray(reference.reference(**inputs))
    err = np.abs(got - want).max() / (np.abs(want).max() + 1e-12)
    print("abs-max relative error:", err)


# revision 30
# speedup vs baseline: 1.0520x; 1.0187x over previous
"""Multi-head GAT Bass kernel for 8 Trainium2 NeuronCores (fp8 edge pipeline).

Sharding: destination-node row-parallel (24 global blocks of 128 rows; core c
owns blocks 3c..3c+2 = 384 output rows). Edges bucketed by destination block
on the host, padded to a uniform per-block count (256-multiple). No
collectives; the host concatenates per-core outputs.

Per-core pipeline (all per-edge tensors fp8):
  stage0: b/a/v projection tables via fp8 DoubleRow matmuls (x, W in fp8,
          weights pre-scaled by 16 for fp8 range; psum f32, cast to fp8).
  edges:  transposed fp8 gathers of a[e0-local] and b[e1] into one tile;
          z = a+b via ONE DoubleRow matmul per slot (identity-pair k-tiles);
          leaky-relu on Act engine (PSUM -> fp8 SBUF) with a parity-
          deinterleaving output AP (the 16-bit-granularity transpose of the
          fp8 gather interleaves features at byte level);
          per-head dot with Wa via J-accumulated DoubleRow matmuls;
          exp on Act; PE transpose; payload p*v on DVE; onehot accumulation
          into psum via group-paired DoubleRow matmuls.
  post:   divide, ELU, residual, LayerNorm (as v1).
"""
import sys
sys.path.insert(0, '/opt/trn_rl_repo')

from contextlib import ExitStack

import numpy as np
import ml_dtypes

import concourse.bass as bass
import concourse.bacc as bacc
import concourse.tile as tile
from concourse import mybir
from concourse.bass_utils import run_bass_kernel_spmd

N = 3072
HID = 512
H = 8
HD = 64
E = 98304
LN_EPS = 1e-5
NCORES = 8
NBLK = 24
BPC = 3
R = 128 * BPC
WS = 16.0     # weight pre-scale for a/b tables (z stored x16)
WAS = 32.0    # wa pre-scale

f32 = mybir.dt.float32
bf16 = mybir.dt.bfloat16
fp8 = mybir.dt.float8e4
f8np = ml_dtypes.float8_e4m3fn
Alu = mybir.AluOpType
Act = mybir.ActivationFunctionType
DR = mybir.MatmulPerfMode.DoubleRow

# fraction of leaky-relu quarters on the DVE engine (rest on Act): k of 8
LEAKY_DVE_K = 0
# b-table evacuation: k of 4 halves on DVE (rest on Act)
BEVAC_DVE_K = 2


def _wrap_idx(idx):
    n = idx.shape[0]
    assert n % 16 == 0
    w16 = idx.reshape(n // 16, 16).T.astype(np.int16)
    return np.ascontiguousarray(np.tile(w16, (8, 1)))


def _ktile(a, k=4):
    """[K, M] -> [128, k, M] with row f = kk*128 + p."""
    K, M = a.shape
    assert K == 128 * k
    return np.ascontiguousarray(a.reshape(k, 128, M).transpose(1, 0, 2))


def prepare(x, edges, Wv, bv, Ww, bw, Wa, ba, gamma, beta):
    e0 = np.asarray(edges[0], np.int64) % N
    e1 = np.asarray(edges[1], np.int64) % N
    blk = e0 >> 7
    order = np.argsort(blk, kind="stable")
    counts = np.bincount(blk, minlength=NBLK)
    B_pad = max(128, int(-(-counts.max() // 128) * 128))
    P = BPC * B_pad

    ga_idx = np.zeros((NBLK, B_pad), np.int16)   # local (within-core) a rows
    gb_idx = np.zeros((NBLK, B_pad), np.int16)
    oh = np.zeros((NBLK, B_pad, 128), np.float32)
    starts = np.zeros(NBLK + 1, np.int64)
    starts[1:] = np.cumsum(counts)
    for b in range(NBLK):
        ids = order[starts[b]:starts[b + 1]]
        c = len(ids)
        core = b // BPC
        ga_idx[b, :c] = e0[ids] - 384 * core
        ga_idx[b, c:] = (b % BPC) * 128
        gb_idx[b, :c] = e1[ids]
        oh[b, np.arange(c), e0[ids] - b * 128] = 1.0

    x = np.asarray(x, np.float32)
    xT = np.ascontiguousarray(x.T)
    xt8 = _ktile(xT).astype(f8np)
    Ww = np.asarray(Ww, np.float32)
    wt8 = _ktile(Ww[:HID] * WS).astype(f8np)
    wb8 = _ktile(Ww[HID:] * WS).astype(f8np)
    wv8 = _ktile(np.asarray(Wv, np.float32)).astype(f8np)
    wa = np.asarray(Wa, np.float32).reshape(2 * HD)

    # wzf [128, 2, 32]: per J-slice [128, 2, 8]: col (2J+u) = wa[2*(p%64)+par]
    # for u == (p >= 64), else 0
    wzf = np.zeros((128, 2, 32), np.float32)
    pr = np.arange(128)
    for par in range(2):
        val = wa[2 * (pr % 64) + par] * WAS
        for J in range(4):
            u = (pr >= 64).astype(np.int64)
            wzf[pr, par, 8 * J + 2 * J + u] = val
    wzf8 = np.ascontiguousarray(wzf.astype(f8np).reshape(128, 64))

    id2 = np.zeros((128, 2, 128), np.float32)
    id2[pr, 0, pr] = 1.0
    id2[pr, 1, pr] = 1.0
    id2_8 = np.ascontiguousarray(id2.astype(f8np).reshape(128, 256))

    bwr = np.ascontiguousarray(np.broadcast_to(bw * WS / 128.0, (128, 2 * HID)).astype(ml_dtypes.bfloat16))
    bvr = np.ascontiguousarray(np.broadcast_to(np.asarray(bv, np.float32) / 128.0, (128, HID)).astype(ml_dtypes.bfloat16))
    gamma_b = np.ascontiguousarray(np.broadcast_to(gamma, (128, HID)).astype(np.float32))
    beta_b = np.ascontiguousarray(np.broadcast_to(beta, (128, HID)).astype(np.float32))
    ba_b = np.full((128, 1), float(np.asarray(ba).reshape(-1)[0]), np.float32)

    in_maps = []
    for c in range(NCORES):
        bs = slice(BPC * c, BPC * (c + 1))
        in_maps.append(dict(
            xt8=xt8,
            xt8o=np.ascontiguousarray(xt8[:, :, R * c:R * (c + 1)]),
            wb8=wb8, wt8=wt8, wv8=wv8,
            bwr=bwr, bvr=bvr,
            gamma_b=gamma_b, beta_b=beta_b, ba_b=ba_b,
            wzf=wzf8, id2=id2_8,
            xs=np.ascontiguousarray(x[R * c:R * (c + 1)]),
            gai=_wrap_idx(ga_idx[bs].reshape(-1)),
            gbi=_wrap_idx(gb_idx[bs].reshape(-1)),
            oh8=np.ascontiguousarray(
                oh[bs].reshape(BPC, B_pad // 128, 128, 128).transpose(2, 0, 1, 3)
                .reshape(128, BPC * (B_pad // 128) * 128).astype(f8np)),
        ))
    return in_maps, B_pad, P


def build(B_pad, P):
    nc = bacc.Bacc("TRN2", target_bir_lowering=False, num_devices=NCORES)

    xt8_in = nc.dram_tensor("xt8", [128, 4 * N], fp8, kind="ExternalInput").ap()
    xt8o_in = nc.dram_tensor("xt8o", [128, 4 * R], fp8, kind="ExternalInput").ap()
    wb8_in = nc.dram_tensor("wb8", [128, 4 * 2 * HID], fp8, kind="ExternalInput").ap()
    wt8_in = nc.dram_tensor("wt8", [128, 4 * 2 * HID], fp8, kind="ExternalInput").ap()
    wv8_in = nc.dram_tensor("wv8", [128, 4 * HID], fp8, kind="ExternalInput").ap()
    bwr_in = nc.dram_tensor("bwr", [128, 2 * HID], bf16, kind="ExternalInput").ap()
    bvr_in = nc.dram_tensor("bvr", [128, HID], bf16, kind="ExternalInput").ap()
    gam_in = nc.dram_tensor("gamma_b", [128, HID], f32, kind="ExternalInput").ap()
    bet_in = nc.dram_tensor("beta_b", [128, HID], f32, kind="ExternalInput").ap()
    ba_in = nc.dram_tensor("ba_b", [128, 1], f32, kind="ExternalInput").ap()
    wzf_in = nc.dram_tensor("wzf", [128, 64], fp8, kind="ExternalInput").ap()
    id2_in = nc.dram_tensor("id2", [128, 256], fp8, kind="ExternalInput").ap()
    xs_in = nc.dram_tensor("xs", [R, HID], f32, kind="ExternalInput").ap()
    gai_in = nc.dram_tensor("gai", [128, P // 16], mybir.dt.int16, kind="ExternalInput").ap()
    gbi_in = nc.dram_tensor("gbi", [128, P // 16], mybir.dt.int16, kind="ExternalInput").ap()
    oh8_in = nc.dram_tensor("oh8", [128, P], fp8, kind="ExternalInput").ap()
    y_out = nc.dram_tensor("y", [R, HID], f32, kind="ExternalOutput").ap()

    a_tbl = nc.dram_tensor("a_tbl", [R, 2 * HID], fp8, kind="Internal").ap()
    b_tbl = nc.dram_tensor("b_tbl", [N, 2 * HID], fp8, kind="Internal").ap()
    v_tbl = nc.dram_tensor("v_tbl", [N, HID], fp8, kind="Internal").ap()

    NT = N // 128
    G = B_pad // 128          # groups per block (even)

    with tile.TileContext(nc, pool_alloc_mode="queue") as tc, ExitStack() as ctx:
        const = ctx.enter_context(tc.tile_pool(name="const", bufs=1))

        # f32 identity for PE transpose of p8
        iota_row = const.tile([128, 128], mybir.dt.int32)
        nc.gpsimd.iota(iota_row[:], pattern=[[1, 128]], base=0, channel_multiplier=0)
        pid = const.tile([128, 1], mybir.dt.int32)
        nc.gpsimd.iota(pid[:], pattern=[[0, 1]], base=0, channel_multiplier=1)
        iota_f = const.tile([128, 128], f32)
        nc.vector.tensor_copy(iota_f[:], iota_row[:])
        pid_f = const.tile([128, 1], f32)
        nc.vector.tensor_copy(pid_f[:], pid[:])
        ident = const.tile([128, 128], f32)
        nc.vector.tensor_scalar(ident[:], iota_f[:], pid_f[:], None, op0=Alu.is_equal)

        gam_sb = const.tile([128, HID], f32)
        nc.sync.dma_start(gam_sb[:], gam_in)
        bet_sb = const.tile([128, HID], f32)
        nc.sync.dma_start(bet_sb[:], bet_in)
        ba_sb = const.tile([128, 1], f32)
        nc.sync.dma_start(ba_sb[:], ba_in)
        wzf_sb = const.tile([128, 2, 32], fp8)
        nc.sync.dma_start(wzf_sb[:], wzf_in.rearrange("p (a b) -> p a b", a=2))
        id2_sb = const.tile([128, 2, 128], fp8)
        nc.sync.dma_start(id2_sb[:], id2_in.rearrange("p (a b) -> p a b", a=2))
        xs_sb = const.tile([128, BPC, HID], f32)
        nc.sync.dma_start(xs_sb[:], xs_in.rearrange("(b p) d -> p b d", p=128))
        gai_sb = const.tile([128, P // 16], mybir.dt.int16)
        nc.sync.dma_start(gai_sb[:], gai_in)
        gbi_sb = const.tile([128, P // 16], mybir.dt.int16)
        nc.sync.dma_start(gbi_sb[:], gbi_in)

        # ---------------- Stage 0: fp8 projection tables ----------------
        with ExitStack() as s0:
            wpool = s0.enter_context(tc.tile_pool(name="wpool", bufs=1))
            s0p = s0.enter_context(tc.tile_pool(name="s0p", bufs=3))
            psum_b = s0.enter_context(tc.tile_pool(name="psum_b", bufs=2, space="PSUM"))
            psum_a = s0.enter_context(tc.tile_pool(name="psum_a", bufs=1, space="PSUM"))
            psum_v = s0.enter_context(tc.tile_pool(name="psum_v", bufs=2, space="PSUM"))

            xt8_sb = wpool.tile([128, 4, N], fp8)
            nc.sync.dma_start(xt8_sb[:], xt8_in.rearrange("p (a n) -> p a n", a=4))
            xt8o_sb = wpool.tile([128, 4, R], fp8)
            nc.sync.dma_start(xt8o_sb[:], xt8o_in.rearrange("p (a n) -> p a n", a=4))
            wb8_sb = wpool.tile([128, 4, 2 * HID], fp8)
            nc.sync.dma_start(wb8_sb[:], wb8_in.rearrange("p (a n) -> p a n", a=4))
            wt8_sb = wpool.tile([128, 4, 2 * HID], fp8)
            nc.sync.dma_start(wt8_sb[:], wt8_in.rearrange("p (a n) -> p a n", a=4))
            wv8_sb = wpool.tile([128, 4, HID], fp8)
            nc.sync.dma_start(wv8_sb[:], wv8_in.rearrange("p (a n) -> p a n", a=4))
            bwr_sb = wpool.tile([128, 2 * HID], bf16)
            nc.sync.dma_start(bwr_sb[:], bwr_in)
            bvr_sb = wpool.tile([128, HID], bf16)
            nc.sync.dma_start(bvr_sb[:], bvr_in)
            ones1 = wpool.tile([128, 128], bf16)
            nc.vector.memset(ones1[:], 1.0)

            # a table first (tiny; gates ga gathers)
            a8 = s0p.tile([128, BPC, 2 * HID], fp8, tag="a8")
            for t in range(BPC):
                psa = psum_a.tile([128, 2 * HID], f32, tag="ps_a")
                for half in range(2):
                    hs = slice(half * HID, (half + 1) * HID)
                    nc.tensor.matmul(psa[:, hs], xt8o_sb[:, 0:2, t * 128:(t + 1) * 128],
                                     wt8_sb[:, 0:2, hs], start=True, stop=False, perf_mode=DR,
                                     skip_group_check=True)
                    nc.tensor.matmul(psa[:, hs], xt8o_sb[:, 2:4, t * 128:(t + 1) * 128],
                                     wt8_sb[:, 2:4, hs], start=False, stop=False, perf_mode=DR,
                                     skip_group_check=True)
                    nc.tensor.matmul(psa[:, hs], ones1[:], bwr_sb[:, hs], start=False,
                                     stop=True, skip_group_check=True)
                    if (t + half) % 2 == 0:
                        nc.vector.tensor_copy(a8[:, t, hs], psa[:, hs])
                    else:
                        nc.scalar.copy(a8[:, t, hs], psa[:, hs])
            nc.sync.dma_start(a_tbl.rearrange("(t p) f -> p t f", p=128), a8[:])
            # b table (gates gb gathers); write in 2-tile batches
            evac = 0
            for nt2 in range(NT // 2):
                b8 = s0p.tile([128, 2, 2 * HID], fp8, tag="b8")
                for k in range(2):
                    nt = 2 * nt2 + k
                    for half in range(2):
                        hs = slice(half * HID, (half + 1) * HID)
                        psb = psum_b.tile([128, HID], f32, tag="ps_b")
                        nc.tensor.matmul(psb[:], xt8_sb[:, 0:2, nt * 128:(nt + 1) * 128],
                                         wb8_sb[:, 0:2, hs], start=True, stop=False, perf_mode=DR)
                        nc.tensor.matmul(psb[:], xt8_sb[:, 2:4, nt * 128:(nt + 1) * 128],
                                         wb8_sb[:, 2:4, hs], start=False, stop=True, perf_mode=DR)
                        if evac % 4 < BEVAC_DVE_K:
                            nc.vector.tensor_copy(b8[:, k, hs], psb[:])
                        else:
                            nc.scalar.copy(b8[:, k, hs], psb[:])
                        evac += 1
                nc.sync.dma_start(
                    b_tbl[nt2 * 256:(nt2 + 1) * 256, :].rearrange("(t p) f -> p t f", p=128),
                    b8[:])
            # v table, 2-tile write batches
            for nt2 in range(NT // 2):
                v8 = s0p.tile([128, 2, HID], fp8, tag="v8")
                for k in range(2):
                    nt = 2 * nt2 + k
                    psv = psum_v.tile([128, HID], f32, tag="ps_v")
                    nc.tensor.matmul(psv[:], xt8_sb[:, 0:2, nt * 128:(nt + 1) * 128],
                                     wv8_sb[:, 0:2, :], start=True, stop=False, perf_mode=DR,
                                     skip_group_check=True)
                    nc.tensor.matmul(psv[:], xt8_sb[:, 2:4, nt * 128:(nt + 1) * 128],
                                     wv8_sb[:, 2:4, :], start=False, stop=False, perf_mode=DR,
                                     skip_group_check=True)
                    nc.tensor.matmul(psv[:], ones1[:], bvr_sb[:], start=False, stop=True,
                                     skip_group_check=True)
                    if nt % 2 == 0:
                        nc.vector.tensor_copy(v8[:, k, :], psv[:])
                    else:
                        nc.scalar.copy(v8[:, k, :], psv[:])
                nc.sync.dma_start(
                    v_tbl[nt2 * 256:(nt2 + 1) * 256, :].rearrange("(t p) f -> p t f", p=128),
                    v8[:])

        # ---------------- Edge stage ----------------
        s12 = ctx.enter_context(tc.tile_pool(name="s12", bufs=4))
        gvp = ctx.enter_context(tc.tile_pool(name="gvp", bufs=2))
        acc = ctx.enter_context(tc.tile_pool(name="acc", bufs=1, space="PSUM"))
        zqp = ctx.enter_context(tc.tile_pool(name="zqp", bufs=2, space="PSUM"))
        pswp = ctx.enter_context(tc.tile_pool(name="pswp", bufs=1, space="PSUM"))
        pstp = ctx.enter_context(tc.tile_pool(name="pstp", bufs=1, space="PSUM"))
        post = ctx.enter_context(tc.tile_pool(name="post", bufs=1))

        chunks = []
        off = 0
        while off < B_pad:
            c = min(512, B_pad - off)
            chunks.append((off, c))
            off += c

        lq = 0
        for blk in range(BPC):
            psum_y = acc.tile([128, HID], f32, tag="psum_y")
            psum_d = acc.tile([128, H], f32, tag="psum_d")
            base = blk * B_pad

            gv_blk = gvp.tile([128, G, HID], fp8, tag="gv")
            oh_blk = gvp.tile([128, G, 128], fp8, tag="ohb")
            nc.sync.dma_start(
                oh_blk[:], oh8_in[:, blk * G * 128:(blk + 1) * G * 128].rearrange(
                    "p (g r) -> p g r", g=G))

            def finish(st):
                p8, coff, C, NG, first, last = st
                pay = s12.tile([128, NG, HID + H], fp8, tag="pay")
                pst = pstp.tile([128, 4, H], f32, tag="pst")
                for g in range(NG):
                    nc.tensor.transpose(pst[:, g, :], p8[:, g * 128:(g + 1) * 128],
                                        ident[:H, :H])
                nc.vector.tensor_copy(pay[:, :, HID:], pst[:, :NG, :])
                nc.vector.tensor_mul(
                    pay[:, :, :HID].rearrange("p c (h d) -> p c h d", h=H),
                    gv_blk[:, coff // 128:coff // 128 + NG, :].rearrange(
                        "p c (h d) -> p c h d", h=H),
                    pay[:, :, HID:].unsqueeze(3).broadcast_to([128, NG, H, HD]))
                oh_c = oh_blk[:, coff // 128:coff // 128 + NG, :]
                rem = NG % 2
                for t in range(NG // 2):
                    st_ = first and t == 0
                    sp = last and rem == 0 and t == NG // 2 - 1
                    nc.tensor.matmul(psum_y[:], oh_c[:, 2 * t:2 * t + 2, :],
                                     pay[:, 2 * t:2 * t + 2, :HID],
                                     start=st_, stop=sp, perf_mode=DR,
                                     skip_group_check=True)
                    nc.tensor.matmul(psum_d[:], oh_c[:, 2 * t:2 * t + 2, :],
                                     pay[:, 2 * t:2 * t + 2, HID:],
                                     start=st_, stop=sp, perf_mode=DR,
                                     skip_group_check=True)
                if rem:
                    st_ = first and NG == 1
                    nc.tensor.matmul(psum_y[:], oh_c[:, NG - 1, :],
                                     pay[:, NG - 1, :HID],
                                     start=st_, stop=last, skip_group_check=True)
                    nc.tensor.matmul(psum_d[:], oh_c[:, NG - 1, :],
                                     pay[:, NG - 1, HID:],
                                     start=st_, stop=last, skip_group_check=True)

            def exp_only(st):
                psw, coff, C, CH, NG, first, last = st
                p8 = s12.tile([8, C], f32, tag="p8")
                nc.scalar.activation(p8[:], psw[:].rearrange("p a c -> p (a c)"),
                                     Act.Exp, bias=ba_sb[:8, :], scale=1.0 / (WS * WAS))
                return (p8, coff, C, NG, first, last)

            prev_dot = None
            prev_fin = None
            for ci, (coff, C) in enumerate(chunks):
                off = base + coff
                i0, i1 = off // 16, (off + C) // 16
                CH = C // 2            # edges per eh-half / cc positions
                NG = C // 128          # groups in chunk

                gab = s12.tile([128, 2, 8, C], fp8, tag="gab")
                nc.gpsimd.dma_gather(
                    out_ap=gab[:, 0], in_ap=a_tbl, idxs_ap=gai_sb[:, i0:i1],
                    num_idxs=C, num_idxs_reg=C, elem_size=2 * HID, transpose=True)
                nc.gpsimd.dma_gather(
                    out_ap=gab[:, 1], in_ap=b_tbl, idxs_ap=gbi_sb[:, i0:i1],
                    num_idxs=C, num_idxs_reg=C, elem_size=2 * HID, transpose=True)
                nv = (B_pad + 1023) // 1024
                if ci < nv:
                    v0 = ci * 1024
                    vc = min(1024, B_pad - v0)
                    nc.gpsimd.dma_gather(
                        out_ap=gv_blk[:, v0 // 128:(v0 + vc) // 128, :], in_ap=v_tbl,
                        idxs_ap=gbi_sb[:, (base + v0) // 16:(base + v0 + vc) // 16],
                        num_idxs=vc, num_idxs_reg=vc, elem_size=HID)

                # z assembly + leaky -> zl2 [128, 8, 2, CH]; dot issued
                # incrementally (J 0-1 after slots 0-3, J 2-3 after 4-7)
                zl2 = s12.tile([128, 8, 2, CH], fp8, tag="zl2")
                psw = pswp.tile([8, 2, CH], f32, tag="psw")
                CW = min(256, C)
                nhalf = max(1, C // 256)
                for sg in range(2):
                    for ch in range(nhalf):
                        zq = zqp.tile([128, 4, 256], f32, tag="zq")
                        for jj in range(4):
                            j = 4 * sg + jj
                            nc.tensor.matmul(
                                zq[:, jj, :CW], id2_sb[:],
                                gab[:, :, j, ch * 256:ch * 256 + CW],
                                start=True, stop=True, perf_mode=DR)
                        dst = zl2[:, 4 * sg:4 * sg + 4, :, ch * 128:ch * 128 + CW // 2]
                        dst = dst.rearrange("p s par cc -> p s cc par")
                        src = zq[:, :, :CW].rearrange("p s (cc par) -> p s cc par", par=2)
                        nc.scalar.activation(dst, src, Act.Prelu, alpha=0.01)
                    for eh in range(2):
                        for J in (0, 1) if sg == 0 else (2, 3):
                            nc.tensor.matmul(
                                psw[:, eh, :], wzf_sb[:, :, 8 * J:8 * J + 8],
                                zl2[:, 2 * J + eh, :, :],
                                start=(J == 0), stop=(J == 3), perf_mode=DR,
                                skip_group_check=True)

                if prev_dot is not None:
                    new_fin = exp_only(prev_dot)
                    if prev_fin is not None:
                        finish(prev_fin)
                    prev_fin = new_fin
                prev_dot = (psw, coff, C, CH, NG, ci == 0, ci == len(chunks) - 1)

            new_fin = exp_only(prev_dot)
            if prev_fin is not None:
                finish(prev_fin)
            finish(new_fin)

            # ---------------- post: divide, ELU, residual, LayerNorm ----------------
            den = post.tile([128, H], f32, tag="den")
            nc.vector.tensor_scalar_add(den[:], psum_d[:], 1e-30)
            rden = post.tile([128, H], f32, tag="rden")
            nc.vector.reciprocal(rden[:], den[:])
            y1 = post.tile([128, HID], f32, tag="y1")
            nc.vector.tensor_mul(
                y1[:].rearrange("p (h d) -> p h d", h=H),
                psum_y[:].rearrange("p (h d) -> p h d", h=H),
                rden[:].unsqueeze(2).broadcast_to([128, H, HD]))
            m1 = post.tile([128, HID], f32, tag="m1")
            nc.vector.tensor_scalar_max(m1[:], y1[:], 0.0)
            t1 = post.tile([128, HID], f32, tag="t1")
            nc.vector.tensor_scalar_min(t1[:], y1[:], 0.0)
            t2 = post.tile([128, HID], f32, tag="t2")
            nc.scalar.activation(t2[:], t1[:], Act.Exp)
            y3 = post.tile([128, HID], f32, tag="y3")
            nc.vector.scalar_tensor_tensor(y3[:], t2[:], -1.0, m1[:],
                                           op0=Alu.add, op1=Alu.add)
            nc.vector.tensor_add(y3[:], y3[:], xs_sb[:, blk, :])
            mu = post.tile([128, 1], f32, tag="mu")
            nc.vector.reduce_sum(mu[:], y3[:], axis=mybir.AxisListType.X)
            nc.vector.tensor_scalar_mul(mu[:], mu[:], 1.0 / HID)
            yc = post.tile([128, HID], f32, tag="yc")
            nc.vector.tensor_scalar(yc[:], y3[:], mu[:], None, op0=Alu.subtract)
            sq = post.tile([128, HID], f32, tag="sq")
            nc.vector.tensor_mul(sq[:], yc[:], yc[:])
            s2 = post.tile([128, 1], f32, tag="s2")
            nc.vector.reduce_sum(s2[:], sq[:], axis=mybir.AxisListType.X)
            var = post.tile([128, 1], f32, tag="var")
            nc.vector.tensor_scalar(var[:], s2[:], 1.0 / HID, LN_EPS,
                                    op0=Alu.mult, op1=Alu.add)
            lnv = post.tile([128, 1], f32, tag="lnv")
            nc.scalar.activation(lnv[:], var[:], Act.Ln)
            rstd = post.tile([128, 1], f32, tag="rstd")
            nc.scalar.activation(rstd[:], lnv[:], Act.Exp, scale=-0.5)
            yn = post.tile([128, HID], f32, tag="yn")
            nc.vector.tensor_scalar(yn[:], yc[:], rstd[:], None, op0=Alu.mult)
            yf = post.tile([128, HID], f32, tag="yf")
            nc.vector.tensor_mul(yf[:], yn[:], gam_sb[:])
            nc.vector.tensor_add(yf[:], yf[:], bet_sb[:])
            nc.sync.dma_start(y_out[blk * 128:(blk + 1) * 128, :], yf[:])

    nc.compile()
    return nc


_CACHE = {}


def get_nc(B_pad, P):
    key = (B_pad, P)
    if key not in _CACHE:
        _CACHE[key] = build(B_pad, P)
    return _CACHE[key]


def kernel(**inputs) -> np.ndarray:
    in_maps, B_pad, P = prepare(**inputs)
    nc = get_nc(B_pad, P)
    res = run_bass_kernel_spmd(nc, in_maps, core_ids=list(range(NCORES)))
    out = np.concatenate([r["y"] for r in res.results], axis=0)
    return out.astype(np.float32)


if __name__ == "__main__":
    import reference
    inputs = {k: np.asarray(v) for k, v in reference.setup_inputs().items()}
    got = kernel(**inputs)
    want = np.asarray(reference.reference(**inputs))
    err = np.abs(got - want).max() / (np.abs(want).max() + 1e-12)
    print("abs-max relative error:", err)


# revision 32
# speedup vs baseline: 1.0532x; 1.0012x over previous
"""Multi-head GAT Bass kernel for 8 Trainium2 NeuronCores (fp8 edge pipeline).

Sharding: destination-node row-parallel (24 global blocks of 128 rows; core c
owns blocks 3c..3c+2 = 384 output rows). Edges bucketed by destination block
on the host, padded to a uniform per-block count (256-multiple). No
collectives; the host concatenates per-core outputs.

Per-core pipeline (all per-edge tensors fp8):
  stage0: b/a/v projection tables via fp8 DoubleRow matmuls (x, W in fp8,
          weights pre-scaled by 16 for fp8 range; psum f32, cast to fp8).
  edges:  transposed fp8 gathers of a[e0-local] and b[e1] into one tile;
          z = a+b via ONE DoubleRow matmul per slot (identity-pair k-tiles);
          leaky-relu on Act engine (PSUM -> fp8 SBUF) with a parity-
          deinterleaving output AP (the 16-bit-granularity transpose of the
          fp8 gather interleaves features at byte level);
          per-head dot with Wa via J-accumulated DoubleRow matmuls;
          exp on Act; PE transpose; payload p*v on DVE; onehot accumulation
          into psum via group-paired DoubleRow matmuls.
  post:   divide, ELU, residual, LayerNorm (as v1).
"""
import sys
sys.path.insert(0, '/opt/trn_rl_repo')

from contextlib import ExitStack

import numpy as np
import ml_dtypes

import concourse.bass as bass
import concourse.bacc as bacc
import concourse.tile as tile
from concourse import mybir
from concourse.bass_utils import run_bass_kernel_spmd

N = 3072
HID = 512
H = 8
HD = 64
E = 98304
LN_EPS = 1e-5
NCORES = 8
NBLK = 24
BPC = 3
R = 128 * BPC
WS = 16.0     # weight pre-scale for a/b tables (z stored x16)
WAS = 32.0    # wa pre-scale

f32 = mybir.dt.float32
bf16 = mybir.dt.bfloat16
fp8 = mybir.dt.float8e4
f8np = ml_dtypes.float8_e4m3fn
Alu = mybir.AluOpType
Act = mybir.ActivationFunctionType
DR = mybir.MatmulPerfMode.DoubleRow

# fraction of leaky-relu quarters on the DVE engine (rest on Act): k of 8
LEAKY_DVE_K = 0
# b-table evacuation: k of 4 halves on DVE (rest on Act)
BEVAC_DVE_K = 2


def _wrap_idx(idx):
    n = idx.shape[0]
    assert n % 16 == 0
    w16 = idx.reshape(n // 16, 16).T.astype(np.int16)
    return np.ascontiguousarray(np.tile(w16, (8, 1)))


def _ktile(a, k=4):
    """[K, M] -> [128, k, M] with row f = kk*128 + p."""
    K, M = a.shape
    assert K == 128 * k
    return np.ascontiguousarray(a.reshape(k, 128, M).transpose(1, 0, 2))


def prepare(x, edges, Wv, bv, Ww, bw, Wa, ba, gamma, beta):
    e0 = np.asarray(edges[0], np.int64) % N
    e1 = np.asarray(edges[1], np.int64) % N
    blk = e0 >> 7
    order = np.argsort(blk, kind="stable")
    counts = np.bincount(blk, minlength=NBLK)
    B_pad = max(128, int(-(-counts.max() // 128) * 128))
    P = BPC * B_pad

    ga_idx = np.zeros((NBLK, B_pad), np.int16)   # local (within-core) a rows
    gb_idx = np.zeros((NBLK, B_pad), np.int16)
    oh = np.zeros((NBLK, B_pad, 128), np.float32)
    starts = np.zeros(NBLK + 1, np.int64)
    starts[1:] = np.cumsum(counts)
    for b in range(NBLK):
        ids = order[starts[b]:starts[b + 1]]
        c = len(ids)
        core = b // BPC
        ga_idx[b, :c] = e0[ids] - 384 * core
        ga_idx[b, c:] = (b % BPC) * 128
        gb_idx[b, :c] = e1[ids]
        oh[b, np.arange(c), e0[ids] - b * 128] = 1.0

    x = np.asarray(x, np.float32)
    xT = np.ascontiguousarray(x.T)
    xt8 = _ktile(xT).astype(f8np)
    Ww = np.asarray(Ww, np.float32)
    wt8 = _ktile(Ww[:HID] * WS).astype(f8np)
    wb8 = _ktile(Ww[HID:] * WS).astype(f8np)
    wv8 = _ktile(np.asarray(Wv, np.float32)).astype(f8np)
    wa = np.asarray(Wa, np.float32).reshape(2 * HD)

    # wzf [128, 2, 32]: per J-slice [128, 2, 8]: col (2J+u) = wa[2*(p%64)+par]
    # for u == (p >= 64), else 0
    wzf = np.zeros((128, 2, 32), np.float32)
    pr = np.arange(128)
    for par in range(2):
        val = wa[2 * (pr % 64) + par] * WAS
        for J in range(4):
            u = (pr >= 64).astype(np.int64)
            wzf[pr, par, 8 * J + 2 * J + u] = val
    wzf8 = np.ascontiguousarray(wzf.astype(f8np).reshape(128, 64))

    id2 = np.zeros((128, 2, 128), np.float32)
    id2[pr, 0, pr] = 1.0
    id2[pr, 1, pr] = 1.0
    id2_8 = np.ascontiguousarray(id2.astype(f8np).reshape(128, 256))

    bwr = np.ascontiguousarray(np.broadcast_to(bw * WS / 128.0, (128, 2 * HID)).astype(ml_dtypes.bfloat16))
    bvr = np.ascontiguousarray(np.broadcast_to(np.asarray(bv, np.float32) / 128.0, (128, HID)).astype(ml_dtypes.bfloat16))
    gamma_b = np.ascontiguousarray(np.broadcast_to(gamma, (128, HID)).astype(np.float32))
    beta_b = np.ascontiguousarray(np.broadcast_to(beta, (128, HID)).astype(np.float32))
    ba_b = np.full((128, 1), float(np.asarray(ba).reshape(-1)[0]), np.float32)

    in_maps = []
    for c in range(NCORES):
        bs = slice(BPC * c, BPC * (c + 1))
        in_maps.append(dict(
            xt8=xt8,
            xt8o=np.ascontiguousarray(xt8[:, :, R * c:R * (c + 1)]),
            wb8=wb8, wt8=wt8, wv8=wv8,
            bwr=bwr, bvr=bvr,
            gamma_b=gamma_b, beta_b=beta_b, ba_b=ba_b,
            wzf=wzf8, id2=id2_8,
            xs=np.ascontiguousarray(x[R * c:R * (c + 1)]),
            gai=_wrap_idx(ga_idx[bs].reshape(-1)),
            gbi=_wrap_idx(gb_idx[bs].reshape(-1)),
            oh8=np.ascontiguousarray(
                oh[bs].reshape(BPC, B_pad // 128, 128, 128).transpose(2, 0, 1, 3)
                .reshape(128, BPC * (B_pad // 128) * 128).astype(f8np)),
        ))
    return in_maps, B_pad, P


def build(B_pad, P):
    nc = bacc.Bacc("TRN2", target_bir_lowering=False, num_devices=NCORES)

    xt8_in = nc.dram_tensor("xt8", [128, 4 * N], fp8, kind="ExternalInput").ap()
    xt8o_in = nc.dram_tensor("xt8o", [128, 4 * R], fp8, kind="ExternalInput").ap()
    wb8_in = nc.dram_tensor("wb8", [128, 4 * 2 * HID], fp8, kind="ExternalInput").ap()
    wt8_in = nc.dram_tensor("wt8", [128, 4 * 2 * HID], fp8, kind="ExternalInput").ap()
    wv8_in = nc.dram_tensor("wv8", [128, 4 * HID], fp8, kind="ExternalInput").ap()
    bwr_in = nc.dram_tensor("bwr", [128, 2 * HID], bf16, kind="ExternalInput").ap()
    bvr_in = nc.dram_tensor("bvr", [128, HID], bf16, kind="ExternalInput").ap()
    gam_in = nc.dram_tensor("gamma_b", [128, HID], f32, kind="ExternalInput").ap()
    bet_in = nc.dram_tensor("beta_b", [128, HID], f32, kind="ExternalInput").ap()
    ba_in = nc.dram_tensor("ba_b", [128, 1], f32, kind="ExternalInput").ap()
    wzf_in = nc.dram_tensor("wzf", [128, 64], fp8, kind="ExternalInput").ap()
    id2_in = nc.dram_tensor("id2", [128, 256], fp8, kind="ExternalInput").ap()
    xs_in = nc.dram_tensor("xs", [R, HID], f32, kind="ExternalInput").ap()
    gai_in = nc.dram_tensor("gai", [128, P // 16], mybir.dt.int16, kind="ExternalInput").ap()
    gbi_in = nc.dram_tensor("gbi", [128, P // 16], mybir.dt.int16, kind="ExternalInput").ap()
    oh8_in = nc.dram_tensor("oh8", [128, P], fp8, kind="ExternalInput").ap()
    y_out = nc.dram_tensor("y", [R, HID], f32, kind="ExternalOutput").ap()

    a_tbl = nc.dram_tensor("a_tbl", [R, 2 * HID], fp8, kind="Internal").ap()
    b_tbl = nc.dram_tensor("b_tbl", [N, 2 * HID], fp8, kind="Internal").ap()
    v_tbl = nc.dram_tensor("v_tbl", [N, HID], fp8, kind="Internal").ap()

    NT = N // 128
    G = B_pad // 128          # groups per block (even)

    with tile.TileContext(nc, pool_alloc_mode="queue") as tc, ExitStack() as ctx:
        const = ctx.enter_context(tc.tile_pool(name="const", bufs=1))

        # f32 identity for PE transpose of p8
        iota_row = const.tile([128, 128], mybir.dt.int32)
        nc.gpsimd.iota(iota_row[:], pattern=[[1, 128]], base=0, channel_multiplier=0)
        pid = const.tile([128, 1], mybir.dt.int32)
        nc.gpsimd.iota(pid[:], pattern=[[0, 1]], base=0, channel_multiplier=1)
        iota_f = const.tile([128, 128], f32)
        nc.vector.tensor_copy(iota_f[:], iota_row[:])
        pid_f = const.tile([128, 1], f32)
        nc.vector.tensor_copy(pid_f[:], pid[:])
        ident = const.tile([128, 128], f32)
        nc.vector.tensor_scalar(ident[:], iota_f[:], pid_f[:], None, op0=Alu.is_equal)

        gam_sb = const.tile([128, HID], f32)
        nc.sync.dma_start(gam_sb[:], gam_in)
        bet_sb = const.tile([128, HID], f32)
        nc.sync.dma_start(bet_sb[:], bet_in)
        ba_sb = const.tile([128, 1], f32)
        nc.sync.dma_start(ba_sb[:], ba_in)
        wzf_sb = const.tile([128, 2, 32], fp8)
        nc.sync.dma_start(wzf_sb[:], wzf_in.rearrange("p (a b) -> p a b", a=2))
        id2_sb = const.tile([128, 2, 128], fp8)
        nc.sync.dma_start(id2_sb[:], id2_in.rearrange("p (a b) -> p a b", a=2))
        xs_sb = const.tile([128, BPC, HID], f32)
        nc.sync.dma_start(xs_sb[:], xs_in.rearrange("(b p) d -> p b d", p=128))
        gai_sb = const.tile([128, P // 16], mybir.dt.int16)
        nc.sync.dma_start(gai_sb[:], gai_in)
        gbi_sb = const.tile([128, P // 16], mybir.dt.int16)
        nc.sync.dma_start(gbi_sb[:], gbi_in)

        # ---------------- Stage 0: fp8 projection tables ----------------
        with ExitStack() as s0:
            wpool = s0.enter_context(tc.tile_pool(name="wpool", bufs=1))
            s0p = s0.enter_context(tc.tile_pool(name="s0p", bufs=3))
            psum_b = s0.enter_context(tc.tile_pool(name="psum_b", bufs=2, space="PSUM"))
            psum_a = s0.enter_context(tc.tile_pool(name="psum_a", bufs=1, space="PSUM"))
            psum_v = s0.enter_context(tc.tile_pool(name="psum_v", bufs=2, space="PSUM"))

            xt8_sb = wpool.tile([128, 4, N], fp8)
            nc.sync.dma_start(xt8_sb[:], xt8_in.rearrange("p (a n) -> p a n", a=4))
            xt8o_sb = wpool.tile([128, 4, R], fp8)
            nc.sync.dma_start(xt8o_sb[:], xt8o_in.rearrange("p (a n) -> p a n", a=4))
            wb8_sb = wpool.tile([128, 4, 2 * HID], fp8)
            nc.sync.dma_start(wb8_sb[:], wb8_in.rearrange("p (a n) -> p a n", a=4))
            wt8_sb = wpool.tile([128, 4, 2 * HID], fp8)
            nc.sync.dma_start(wt8_sb[:], wt8_in.rearrange("p (a n) -> p a n", a=4))
            wv8_sb = wpool.tile([128, 4, HID], fp8)
            nc.sync.dma_start(wv8_sb[:], wv8_in.rearrange("p (a n) -> p a n", a=4))
            bwr_sb = wpool.tile([128, 2 * HID], bf16)
            nc.sync.dma_start(bwr_sb[:], bwr_in)
            bvr_sb = wpool.tile([128, HID], bf16)
            nc.sync.dma_start(bvr_sb[:], bvr_in)
            ones1 = wpool.tile([128, 128], bf16)
            nc.vector.memset(ones1[:], 1.0)

            # a table first (tiny; gates ga gathers)
            a8 = s0p.tile([128, BPC, 2 * HID], fp8, tag="a8")
            for t in range(BPC):
                psa = psum_a.tile([128, 2 * HID], f32, tag="ps_a")
                for half in range(2):
                    hs = slice(half * HID, (half + 1) * HID)
                    nc.tensor.matmul(psa[:, hs], xt8o_sb[:, 0:2, t * 128:(t + 1) * 128],
                                     wt8_sb[:, 0:2, hs], start=True, stop=False, perf_mode=DR,
                                     skip_group_check=True)
                    nc.tensor.matmul(psa[:, hs], xt8o_sb[:, 2:4, t * 128:(t + 1) * 128],
                                     wt8_sb[:, 2:4, hs], start=False, stop=False, perf_mode=DR,
                                     skip_group_check=True)
                    nc.tensor.matmul(psa[:, hs], ones1[:], bwr_sb[:, hs], start=False,
                                     stop=True, skip_group_check=True)
                    if (t + half) % 2 == 0:
                        nc.vector.tensor_copy(a8[:, t, hs], psa[:, hs])
                    else:
                        nc.scalar.copy(a8[:, t, hs], psa[:, hs])
            nc.sync.dma_start(a_tbl.rearrange("(t p) f -> p t f", p=128), a8[:])
            # b table (gates gb gathers); write in 2-tile batches
            evac = 0
            for nt2 in range(NT // 2):
                b8 = s0p.tile([128, 2, 2 * HID], fp8, tag="b8")
                for k in range(2):
                    nt = 2 * nt2 + k
                    for half in range(2):
                        hs = slice(half * HID, (half + 1) * HID)
                        psb = psum_b.tile([128, HID], f32, tag="ps_b")
                        nc.tensor.matmul(psb[:], xt8_sb[:, 0:2, nt * 128:(nt + 1) * 128],
                                         wb8_sb[:, 0:2, hs], start=True, stop=False, perf_mode=DR)
                        nc.tensor.matmul(psb[:], xt8_sb[:, 2:4, nt * 128:(nt + 1) * 128],
                                         wb8_sb[:, 2:4, hs], start=False, stop=True, perf_mode=DR)
                        if evac % 4 < BEVAC_DVE_K:
                            nc.vector.tensor_copy(b8[:, k, hs], psb[:])
                        else:
                            nc.scalar.copy(b8[:, k, hs], psb[:])
                        evac += 1
                nc.sync.dma_start(
                    b_tbl[nt2 * 256:(nt2 + 1) * 256, :].rearrange("(t p) f -> p t f", p=128),
                    b8[:])
            # v table, 2-tile write batches
            for nt2 in range(NT // 2):
                v8 = s0p.tile([128, 2, HID], fp8, tag="v8")
                for k in range(2):
                    nt = 2 * nt2 + k
                    psv = psum_v.tile([128, HID], f32, tag="ps_v")
                    nc.tensor.matmul(psv[:], xt8_sb[:, 0:2, nt * 128:(nt + 1) * 128],
                                     wv8_sb[:, 0:2, :], start=True, stop=False, perf_mode=DR,
                                     skip_group_check=True)
                    nc.tensor.matmul(psv[:], xt8_sb[:, 2:4, nt * 128:(nt + 1) * 128],
                                     wv8_sb[:, 2:4, :], start=False, stop=False, perf_mode=DR,
                                     skip_group_check=True)
                    nc.tensor.matmul(psv[:], ones1[:], bvr_sb[:], start=False, stop=True,
                                     skip_group_check=True)
                    if nt % 2 == 0:
                        nc.vector.tensor_copy(v8[:, k, :], psv[:])
                    else:
                        nc.scalar.copy(v8[:, k, :], psv[:])
                nc.sync.dma_start(
                    v_tbl[nt2 * 256:(nt2 + 1) * 256, :].rearrange("(t p) f -> p t f", p=128),
                    v8[:])

        # ---------------- Edge stage ----------------
        s12 = ctx.enter_context(tc.tile_pool(name="s12", bufs=5))
        gvp = ctx.enter_context(tc.tile_pool(name="gvp", bufs=2))
        acc = ctx.enter_context(tc.tile_pool(name="acc", bufs=1, space="PSUM"))
        zqp = ctx.enter_context(tc.tile_pool(name="zqp", bufs=2, space="PSUM"))
        pswp = ctx.enter_context(tc.tile_pool(name="pswp", bufs=1, space="PSUM"))
        pstp = ctx.enter_context(tc.tile_pool(name="pstp", bufs=1, space="PSUM"))
        post = ctx.enter_context(tc.tile_pool(name="post", bufs=1))

        chunks = []
        off = 0
        while off < B_pad:
            c = min(512, B_pad - off)
            chunks.append((off, c))
            off += c

        lq = 0
        for blk in range(BPC):
            psum_y = acc.tile([128, HID], f32, tag="psum_y")
            psum_d = acc.tile([128, H], f32, tag="psum_d")
            base = blk * B_pad

            gv_blk = gvp.tile([128, G, HID], fp8, tag="gv")
            oh_blk = gvp.tile([128, G, 128], fp8, tag="ohb")
            nc.sync.dma_start(
                oh_blk[:], oh8_in[:, blk * G * 128:(blk + 1) * G * 128].rearrange(
                    "p (g r) -> p g r", g=G))

            def finish(st):
                p8, coff, C, NG, first, last = st
                pay = s12.tile([128, NG, HID + H], fp8, tag="pay")
                pst = pstp.tile([128, 4, H], f32, tag="pst")
                for g in range(NG):
                    nc.tensor.transpose(pst[:, g, :], p8[:, g * 128:(g + 1) * 128],
                                        ident[:H, :H])
                nc.vector.tensor_copy(pay[:, :, HID:], pst[:, :NG, :])
                nc.vector.tensor_mul(
                    pay[:, :, :HID].rearrange("p c (h d) -> p c h d", h=H),
                    gv_blk[:, coff // 128:coff // 128 + NG, :].rearrange(
                        "p c (h d) -> p c h d", h=H),
                    pay[:, :, HID:].unsqueeze(3).broadcast_to([128, NG, H, HD]))
                oh_c = oh_blk[:, coff // 128:coff // 128 + NG, :]
                rem = NG % 2
                for t in range(NG // 2):
                    st_ = first and t == 0
                    sp = last and rem == 0 and t == NG // 2 - 1
                    nc.tensor.matmul(psum_y[:], oh_c[:, 2 * t:2 * t + 2, :],
                                     pay[:, 2 * t:2 * t + 2, :HID],
                                     start=st_, stop=sp, perf_mode=DR,
                                     skip_group_check=True)
                    nc.tensor.matmul(psum_d[:], oh_c[:, 2 * t:2 * t + 2, :],
                                     pay[:, 2 * t:2 * t + 2, HID:],
                                     start=st_, stop=sp, perf_mode=DR,
                                     skip_group_check=True)
                if rem:
                    st_ = first and NG == 1
                    nc.tensor.matmul(psum_y[:], oh_c[:, NG - 1, :],
                                     pay[:, NG - 1, :HID],
                                     start=st_, stop=last, skip_group_check=True)
                    nc.tensor.matmul(psum_d[:], oh_c[:, NG - 1, :],
                                     pay[:, NG - 1, HID:],
                                     start=st_, stop=last, skip_group_check=True)

            def exp_only(st):
                psw, coff, C, CH, NG, first, last = st
                p8 = s12.tile([8, C], f32, tag="p8")
                nc.scalar.activation(p8[:], psw[:].rearrange("p a c -> p (a c)"),
                                     Act.Exp, bias=ba_sb[:8, :], scale=1.0 / (WS * WAS))
                return (p8, coff, C, NG, first, last)

            prev_dot = None
            prev_fin = None
            for ci, (coff, C) in enumerate(chunks):
                off = base + coff
                i0, i1 = off // 16, (off + C) // 16
                CH = C // 2            # edges per eh-half / cc positions
                NG = C // 128          # groups in chunk

                gab = s12.tile([128, 2, 8, C], fp8, tag="gab")
                nc.gpsimd.dma_gather(
                    out_ap=gab[:, 0], in_ap=a_tbl, idxs_ap=gai_sb[:, i0:i1],
                    num_idxs=C, num_idxs_reg=C, elem_size=2 * HID, transpose=True)
                nc.gpsimd.dma_gather(
                    out_ap=gab[:, 1], in_ap=b_tbl, idxs_ap=gbi_sb[:, i0:i1],
                    num_idxs=C, num_idxs_reg=C, elem_size=2 * HID, transpose=True)
                nv = (B_pad + 1023) // 1024
                if ci < nv:
                    v0 = ci * 1024
                    vc = min(1024, B_pad - v0)
                    nc.gpsimd.dma_gather(
                        out_ap=gv_blk[:, v0 // 128:(v0 + vc) // 128, :], in_ap=v_tbl,
                        idxs_ap=gbi_sb[:, (base + v0) // 16:(base + v0 + vc) // 16],
                        num_idxs=vc, num_idxs_reg=vc, elem_size=HID)

                # z assembly + leaky -> zl2 [128, 8, 2, CH]; dot issued
                # incrementally (J 0-1 after slots 0-3, J 2-3 after 4-7)
                zl2 = s12.tile([128, 8, 2, CH], fp8, tag="zl2")
                psw = pswp.tile([8, 2, CH], f32, tag="psw")
                CW = min(256, C)
                nhalf = max(1, C // 256)
                for sg in range(2):
                    for ch in range(nhalf):
                        zq = zqp.tile([128, 4, 256], f32, tag="zq")
                        for jj in range(4):
                            j = 4 * sg + jj
                            nc.tensor.matmul(
                                zq[:, jj, :CW], id2_sb[:],
                                gab[:, :, j, ch * 256:ch * 256 + CW],
                                start=True, stop=True, perf_mode=DR)
                        dst = zl2[:, 4 * sg:4 * sg + 4, :, ch * 128:ch * 128 + CW // 2]
                        dst = dst.rearrange("p s par cc -> p s cc par")
                        src = zq[:, :, :CW].rearrange("p s (cc par) -> p s cc par", par=2)
                        nc.scalar.activation(dst, src, Act.Prelu, alpha=0.01)
                    for eh in range(2):
                        for J in (0, 1) if sg == 0 else (2, 3):
                            nc.tensor.matmul(
                                psw[:, eh, :], wzf_sb[:, :, 8 * J:8 * J + 8],
                                zl2[:, 2 * J + eh, :, :],
                                start=(J == 0), stop=(J == 3), perf_mode=DR,
                                skip_group_check=True)

                if prev_dot is not None:
                    new_fin = exp_only(prev_dot)
                    if prev_fin is not None:
                        finish(prev_fin)
                    prev_fin = new_fin
                prev_dot = (psw, coff, C, CH, NG, ci == 0, ci == len(chunks) - 1)

            new_fin = exp_only(prev_dot)
            if prev_fin is not None:
                finish(prev_fin)
            finish(new_fin)

            # ---------------- post: divide, ELU, residual, LayerNorm ----------------
            den = post.tile([128, H], f32, tag="den")
            nc.vector.tensor_scalar_add(den[:], psum_d[:], 1e-30)
            rden = post.tile([128, H], f32, tag="rden")
            nc.vector.reciprocal(rden[:], den[:])
            y1 = post.tile([128, HID], f32, tag="y1")
            nc.vector.tensor_mul(
                y1[:].rearrange("p (h d) -> p h d", h=H),
                psum_y[:].rearrange("p (h d) -> p h d", h=H),
                rden[:].unsqueeze(2).broadcast_to([128, H, HD]))
            m1 = post.tile([128, HID], f32, tag="m1")
            nc.vector.tensor_scalar_max(m1[:], y1[:], 0.0)
            t1 = post.tile([128, HID], f32, tag="t1")
            nc.vector.tensor_scalar_min(t1[:], y1[:], 0.0)
            t2 = post.tile([128, HID], f32, tag="t2")
            nc.scalar.activation(t2[:], t1[:], Act.Exp)
            y3 = post.tile([128, HID], f32, tag="y3")
            nc.vector.scalar_tensor_tensor(y3[:], t2[:], -1.0, m1[:],
                                           op0=Alu.add, op1=Alu.add)
            nc.vector.tensor_add(y3[:], y3[:], xs_sb[:, blk, :])
            mu = post.tile([128, 1], f32, tag="mu")
            nc.vector.reduce_sum(mu[:], y3[:], axis=mybir.AxisListType.X)
            nc.vector.tensor_scalar_mul(mu[:], mu[:], 1.0 / HID)
            yc = post.tile([128, HID], f32, tag="yc")
            nc.vector.tensor_scalar(yc[:], y3[:], mu[:], None, op0=Alu.subtract)
            sq = post.tile([128, HID], f32, tag="sq")
            nc.vector.tensor_mul(sq[:], yc[:], yc[:])
            s2 = post.tile([128, 1], f32, tag="s2")
            nc.vector.reduce_sum(s2[:], sq[:], axis=mybir.AxisListType.X)
            var = post.tile([128, 1], f32, tag="var")
            nc.vector.tensor_scalar(var[:], s2[:], 1.0 / HID, LN_EPS,
                                    op0=Alu.mult, op1=Alu.add)
            lnv = post.tile([128, 1], f32, tag="lnv")
            nc.scalar.activation(lnv[:], var[:], Act.Ln)
            rstd = post.tile([128, 1], f32, tag="rstd")
            nc.scalar.activation(rstd[:], lnv[:], Act.Exp, scale=-0.5)
            yn = post.tile([128, HID], f32, tag="yn")
            nc.vector.tensor_scalar(yn[:], yc[:], rstd[:], None, op0=Alu.mult)
            yf = post.tile([128, HID], f32, tag="yf")
            nc.vector.tensor_mul(yf[:], yn[:], gam_sb[:])
            nc.vector.tensor_add(yf[:], yf[:], bet_sb[:])
            nc.sync.dma_start(y_out[blk * 128:(blk + 1) * 128, :], yf[:])

    nc.compile()
    return nc


_CACHE = {}


def get_nc(B_pad, P):
    key = (B_pad, P)
    if key not in _CACHE:
        _CACHE[key] = build(B_pad, P)
    return _CACHE[key]


def kernel(**inputs) -> np.ndarray:
    in_maps, B_pad, P = prepare(**inputs)
    nc = get_nc(B_pad, P)
    res = run_bass_kernel_spmd(nc, in_maps, core_ids=list(range(NCORES)))
    out = np.concatenate([r["y"] for r in res.results], axis=0)
    return out.astype(np.float32)


if __name__ == "__main__":
    import reference
    inputs = {k: np.asarray(v) for k, v in reference.setup_inputs().items()}
    got = kernel(**inputs)
    want = np.asarray(reference.reference(**inputs))
    err = np.abs(got - want).max() / (np.abs(want).max() + 1e-12)
    print("abs-max relative error:", err)
